# revision 13
# baseline (speedup 1.0000x reference)
"""BasedAttention Trainium2 kernel — nn_BasedAttention_82214263980185.

Head-sharded across 8 NeuronCores (2 heads/core): column-parallel QKV,
per-head taylor linear attention (factorized phi) + banded sliding-window
attention, row-parallel out-proj with host-side partial reduction.

Math notes:
  - reference phi(x) = [1, x, tri-scaled quad] gives
    phi(q).phi(k) = 1 + s + 0.25 s^2  (s = qf.kf).  We use the equivalent
    full-outer 256-feature quad block scaled 2^-0.25 per side plus
    [x, ones]: identical inner products, rectangular construction.
  - Intra-chunk scores: A = (1 + 0.5 s)^2 = 1 + s + 0.25 s^2 directly.
  - rmsnorm: norm_w folds into QKV weights on host; the per-row 1/rms
    factor r applies to q, k, v after projection (all linear in r).
"""

import math
import sys

for _p in ("/opt/trn_rl_repo",):
    if _p not in sys.path:
        sys.path.insert(0, _p)

import numpy as np
import ml_dtypes

import concourse.bass as bass
import concourse.mybir as mybir
import concourse.tile as tile
from concourse.bass_utils import run_bass_kernel_spmd

F32 = mybir.dt.float32
BF16 = mybir.dt.bfloat16
AF = mybir.ActivationFunctionType
ALU = mybir.AluOpType
BF = ml_dtypes.bfloat16

B, T, D = 2, 2048, 1024
P = B * T          # 4096 positions
NH, DH, FT = 16, 64, 16
HPC = 2            # heads per core
NCORES = 8
WINDOW = 64
EPS_NORM = 1e-6
EPS_LIN = 1e-6
SUB = 128          # position sub-chunk (partition tile)
NSUB = P // SUB    # 32
SC = 256           # linear-attention scan chunk
NSC_B = T // SC    # 8 scan chunks per (b,h) sequence
QK_SCALE = 1.0 / math.sqrt(DH)
QUAD_PRE = 2.0 ** (-0.25)


def _fix_tile_drain():
    """walrus here accepts only 1 sync-wait on the Tile tail drain; spread
    the global-clock waits over sequencer nop carriers."""
    if getattr(tile.TileContext, "_drain_fix", False):
        return
    from concourse.tile import ScopedClock

    def _patched(self, tick_clock, wait_clock):
        nc = self.nc
        carriers = [nc.sync.nop(nofuse=True) for _ in range(30)]
        drain_inst = nc.sync.drain()
        wait_clock.add_sem_waits(
            drain_inst.ins, ScopedClock({None: tick_clock.global_clock})
        )
        si = drain_inst.ins.sync_info
        waits = list(si.on_wait) if si is not None else []
        if len(waits) > 1:
            keep, rest = waits[:1], waits[1:]
            assert len(rest) <= len(carriers), f"too many waits: {len(waits)}"
            for c, w in zip(carriers, rest):
                c.ins.sync_info = mybir.SyncInfo(on_wait=[w], on_update=[])
            drain_inst.ins.sync_info = mybir.SyncInfo(
                on_wait=keep, on_update=list(si.on_update)
            )
        nc.all_engine_barrier()
        assert self.sems is not None
        popped = nc._tile_sem_poison_stack.pop()
        assert popped is self._sem_poison
        nc.clear_and_free_semaphores(list(self.sems.allocated().values()))
        nc.all_engine_barrier()

    tile.TileContext._drain_and_barrier = _patched
    tile.TileContext._drain_fix = True


def _split_excess_waits(nc, limit=1):
    """walrus in this container rejects instructions with more than one
    embedded sync-wait; hoist excess waits onto preceding same-engine nops."""
    n = 0
    for f in nc.m.functions:
        for b in f.blocks:
            insts = b.instructions
            out = []
            changed = False
            for ins in insts:
                si = ins.sync_info
                waits = list(si.on_wait) if si is not None else []
                if len(waits) > limit:
                    changed = True
                    for w in waits[:-limit]:
                        n += 1
                        out.append(mybir.InstNoOp(
                            name=f"waitnop-{n}", engine=ins.engine,
                            bass_nofuse=True,
                            sync_info=mybir.SyncInfo(on_wait=[w],
                                                     on_update=[])))
                    ins.sync_info = mybir.SyncInfo(
                        on_wait=waits[-limit:], on_update=list(si.on_update))
                out.append(ins)
            if changed:
                b.instructions = out
    return n


def build_bass():
    _fix_tile_drain()
    nc = bass.Bass()
    dram = {}
    for name, shape in [
        ("xT", [D, P]), ("wq", [D, 128]), ("wk", [D, 128]), ("wv", [D, 128]),
        ("wfq", [128, 49]), ("wfk", [128, 49]),
        ("wqf1", [128, FT]), ("wkf1", [128, FT]),
        ("w1", [128, D]), ("w2", [128, D]),
        ("mtri", [128, 128]), ("mwd", [128, 128]), ("mwp", [128, 128]),
        ("onesP", [1, P]),
        ("ident", [128, 128]),
    ]:
        dram[name] = nc.dram_tensor(name, shape, BF16, kind="ExternalInput")
    dram["out"] = nc.dram_tensor("out", [P, D], BF16, kind="ExternalOutput")
    dram["scr1"] = nc.dram_tensor("scr1", [P], F32)
    dram["scr2"] = nc.dram_tensor("scr2", [P], F32)
    with tile.TileContext(nc) as tc:
        _emit(nc, tc, dram)
    _split_excess_waits(nc)
    return nc


def _emit(nc, tc, dram):
    from contextlib import ExitStack

    with ExitStack() as ctx:
        const = ctx.enter_context(tc.tile_pool(name="const", bufs=1))
        big = ctx.enter_context(tc.tile_pool(name="big", bufs=1))
        work = ctx.enter_context(tc.tile_pool(name="work", bufs=3))

        # ---- constants -----------------------------------------------
        cs = {}
        for name in ("ident", "mtri", "mwd", "mwp",
                     "wfq", "wfk", "wqf1", "wkf1", "w1", "w2"):
            d = dram[name]
            t_ = const.tile(list(d.shape), BF16, tag=name)
            nc.sync.dma_start(t_[:], d[:])
            cs[name] = t_
        for name in ("wq", "wk", "wv"):
            d = dram[name]
            t_ = const.tile([128, 8 * 128], BF16, tag=name)
            for kk in range(8):
                nc.sync.dma_start(t_[:, kk * 128:(kk + 1) * 128],
                                  d[kk * 128:(kk + 1) * 128, :])
            cs[name] = t_
        ones_col_b = const.tile([128, 1], BF16, tag="ocb")
        nc.gpsimd.memset(ones_col_b[:], 1.0)
        ones64_f = const.tile([1, 64], F32, tag="o64")
        nc.gpsimd.memset(ones64_f[:], 1.0)
        ones128_f = const.tile([1, 128], F32, tag="o128")
        nc.gpsimd.memset(ones128_f[:], 1.0)
        epsn_col = const.tile([128, 1], F32, tag="epsn")
        nc.gpsimd.memset(epsn_col[:], EPS_NORM)

        # ---- big persistent tiles ------------------------------------
        qT = big.tile([128, P], BF16, tag="qT")
        kT = big.tile([128, P], BF16, tag="kT")
        Vt = big.tile([128, NSUB * 130], BF16, tag="Vt")
        qfT = big.tile([64, P], BF16, tag="qfT")   # rows 16, 48 = ones
        kfT = big.tile([64, P], BF16, tag="kfT")
        catL = big.tile([128, P], BF16, tag="catL")
        catW = big.tile([128, P], BF16, tag="catW")
        r32 = big.tile([128, NSUB], F32, tag="r32")
        r_row = big.tile([1, P], F32, tag="rrow")
        sq_row = big.tile([1, P], F32, tag="sqrow")

        def vsl(gsub, h):
            base = gsub * 130 + 65 * h
            return Vt[:, base:base + 65]

        with tc.tile_pool(name="xp", bufs=1) as xp:
            xt_sb = xp.tile([128, 8 * P], BF16, tag="xt")
            xv = [xt_sb[:, kk * P:(kk + 1) * P] for kk in range(8)]
            for kk in range(8):
                nc.sync.dma_start(xv[kk],
                                  dram["xT"][kk * 128:(kk + 1) * 128, :])

            # ---- rmsnorm scale r -------------------------------------
            with tc.tile_pool(name="psq", bufs=1, space="PSUM") as psq:
                for pc in range(8):
                    sl = slice(pc * 512, (pc + 1) * 512)
                    sq_ps = psq.tile([1, 512], F32, tag="sqps")
                    for kk in range(8):
                        sqt = work.tile([128, 512], BF16, tag="sq")
                        src = xv[kk][:, sl]
                        if kk % 2 == 0:
                            nc.scalar.activation(sqt[:], src, AF.Square)
                        else:
                            nc.vector.tensor_tensor(sqt[:], src, src, ALU.mult)
                        nc.tensor.matmul(sq_ps[:], ones_col_b[:], sqt[:],
                                         start=(kk == 0), stop=(kk == 7))
                    nc.scalar.copy(sq_row[0:1, sl], sq_ps[:])
            # (1,P) -> (128,32) via DRAM bounce: r32[o, s] = row[s*128+o]
            nc.sync.dma_start(dram["scr1"][:], sq_row[:])
            nc.sync.dma_start(
                r32[:], dram["scr1"][:].rearrange("(s o) -> o s", o=128))
            nc.scalar.activation(r32[:], r32[:], AF.Sqrt,
                                 bias=epsn_col[:], scale=1.0 / D)
            nc.vector.reciprocal(r32[:], r32[:])
            nc.sync.dma_start(
                dram["scr2"][:].rearrange("(s o) -> o s", o=128), r32[:])
            nc.sync.dma_start(r_row[:], dram["scr2"][:])

            # ---- q/k projections (d-part) ----------------------------
            wq8 = [cs["wq"][:, kk * 128:(kk + 1) * 128] for kk in range(8)]
            wk8 = [cs["wk"][:, kk * 128:(kk + 1) * 128] for kk in range(8)]
            wv8 = [cs["wv"][:, kk * 128:(kk + 1) * 128] for kk in range(8)]
            with tc.tile_pool(name="ppj", bufs=2, space="PSUM") as ppj:
                for pc in range(8):
                    sl = slice(pc * 512, (pc + 1) * 512)
                    rb_ps = ppj.tile([128, 512], F32, tag="rb")
                    nc.tensor.matmul(rb_ps[:], ones128_f[:], r_row[0:1, sl],
                                     start=True, stop=True)
                    rb_sb = work.tile([128, 512], F32, tag="rbsb")
                    nc.scalar.copy(rb_sb[:], rb_ps[:])
                    for dst, w8 in ((qT, wq8), (kT, wk8)):
                        pj = ppj.tile([128, 512], F32, tag="pj")
                        for kk in range(8):
                            nc.tensor.matmul(pj[:], w8[kk], xv[kk][:, sl],
                                             start=(kk == 0), stop=(kk == 7))
                        nc.vector.tensor_tensor(dst[:, sl], pj[:], rb_sb[:],
                                                ALU.mult)

            # ---- V (pos-part, r-scaled, ones col) --------------------
            with tc.tile_pool(name="pv", bufs=2, space="PSUM") as pv:
                for s in range(NSUB):
                    sl = slice(s * SUB, (s + 1) * SUB)
                    vp = pv.tile([128, 128], F32, tag="vp")
                    for kk in range(8):
                        nc.tensor.matmul(vp[:], xv[kk][:, sl], wv8[kk],
                                         start=(kk == 0), stop=(kk == 7))
                    rcol = r32[:, s:s + 1]
                    for h in range(HPC):
                        va = vsl(s, h)
                        nc.vector.tensor_scalar_mul(
                            va[:, 0:64], vp[:, 64 * h:64 * h + 64], rcol)
                        nc.gpsimd.memset(va[:, 64:65], 1.0)

        # ---- qfT / kfT (17-part per head, rows 16/33 ones) -----------
        with tc.tile_pool(name="pf", bufs=2, space="PSUM") as pf:
            for pc in range(8):
                sl = slice(pc * 512, (pc + 1) * 512)
                for dst, wf, src in ((qfT, cs["wfq"], qT), (kfT, cs["wfk"], kT)):
                    fp = pf.tile([49, 512], F32, tag="fp")
                    nc.tensor.matmul(fp[:], wf[:], src[:, sl],
                                     start=True, stop=True)
                    nc.vector.tensor_copy(dst[0:49, sl], fp[:])
        nc.sync.dma_start(qfT[16:17, :], dram["onesP"][:])
        nc.sync.dma_start(qfT[48:49, :], dram["onesP"][:])

        # ---- linear attention scan -----------------------------------
        with tc.tile_pool(name="pkv", bufs=1, space="PSUM") as pkv, \
             tc.tile_pool(name="psc", bufs=1, space="PSUM") as psc, \
             tc.tile_pool(name="pyt", bufs=2, space="PSUM") as pyt, \
             tc.tile_pool(name="phi", bufs=3) as phip:
            for b in range(B):
                for h in range(HPC):
                    hd = slice(h * DH, (h + 1) * DH)
                    h17 = slice(h * 32, h * 32 + 17)
                    h16 = slice(h * 32, h * 32 + 16)
                    kvq = pkv.tile([128, 130], F32, tag="kvq")
                    kvlo = pkv.tile([17, 65], F32, tag="kvlo")
                    kvq_sb = work.tile([128, 130], BF16, tag="kvqs")
                    kvlo_sb = work.tile([49, 65], BF16, tag="kvlos")
                    for sc in range(NSC_B):
                        p0 = b * T + sc * SC
                        gs0 = p0 // SUB
                        quads_q, quads_k = [], []
                        for cb in range(2):
                            sl = slice(p0 + cb * 128, p0 + (cb + 1) * 128)
                            qk_ps = psc.tile([128, 32], F32, tag="qkps")
                            nc.tensor.matmul(qk_ps[:, 0:16], qT[hd, sl],
                                             cs["wqf1"][hd, :], start=True,
                                             stop=True)
                            nc.tensor.matmul(qk_ps[:, 16:32], kT[hd, sl],
                                             cs["wkf1"][hd, :], start=True,
                                             stop=True)
                            qfp = phip.tile([128, FT], BF16, tag="qfp")
                            kfp = phip.tile([128, FT], BF16, tag="kfp")
                            klin = phip.tile([128, 17], BF16, tag="klin")
                            nc.scalar.activation(qfp[:], qk_ps[:, 0:16],
                                                 AF.Copy, bias=0.0,
                                                 scale=QUAD_PRE)
                            nc.scalar.activation(kfp[:], qk_ps[:, 16:32],
                                                 AF.Copy, bias=0.0,
                                                 scale=QUAD_PRE)
                            nc.scalar.copy(klin[:, 0:16], qk_ps[:, 16:32])
                            nc.gpsimd.memset(klin[:, 16:17], 1.0)
                            quad_q = phip.tile([128, 256], BF16, tag="qq")
                            quad_k = phip.tile([128, 256], BF16, tag="qk")
                            for qd, fsrc in ((quad_q, qfp), (quad_k, kfp)):
                                g1 = fsrc[:].unsqueeze(2).broadcast_to(
                                    (128, FT, FT))
                                g2 = fsrc[:].unsqueeze(1).broadcast_to(
                                    (128, FT, FT))
                                nc.vector.tensor_tensor(
                                    qd[:].rearrange("p (i j) -> p i j", i=FT),
                                    g1, g2, ALU.mult)
                            q1sb = phip.tile([128, 128], BF16, tag="q1sb")
                            q2sb = phip.tile([128, 128], BF16, tag="q2sb")
                            for half, qsb in ((0, q1sb), (1, q2sb)):
                                tq = psc.tile([128, 128], BF16, tag="tq")
                                nc.tensor.transpose(
                                    tq[:],
                                    quad_q[:, half * 128:(half + 1) * 128],
                                    cs["ident"][:])
                                nc.scalar.copy(qsb[:], tq[:])
                            quads_q.append((q1sb, q2sb))
                            quads_k.append((quad_k, klin))

                        yts = []
                        for cb in range(2):
                            sl = slice(p0 + cb * 128, p0 + (cb + 1) * 128)
                            yt = pyt.tile([65, 128], F32, tag="yt")
                            ops = []
                            for sb in range(cb + 1):
                                ssl = slice(p0 + sb * 128,
                                            p0 + (sb + 1) * 128)
                                s_ps = psc.tile([128, 128], F32, tag="sps")
                                nc.tensor.matmul(s_ps[:], kfT[h16, ssl],
                                                 qfT[h16, sl],
                                                 start=True, stop=True)
                                a_sb = work.tile([128, 128], BF16, tag="asb")
                                nc.scalar.activation(a_sb[:], s_ps[:],
                                                     AF.Square,
                                                     bias=1.0, scale=0.5)
                                if sb == cb:
                                    nc.vector.tensor_tensor(
                                        a_sb[:], a_sb[:], cs["mtri"][:],
                                        ALU.mult)
                                ops.append((vsl(gs0 + sb, h), a_sb[:]))
                            if sc > 0:
                                q1sb, q2sb = quads_q[cb]
                                ops.append((kvq_sb[:, 0:65], q1sb[:]))
                                ops.append((kvq_sb[:, 65:130], q2sb[:]))
                                ops.append((kvlo_sb[h17, :], qfT[h17, sl]))
                            for i, (lt, rt) in enumerate(ops):
                                nc.tensor.matmul(yt[:], lt, rt,
                                                 start=(i == 0),
                                                 stop=(i == len(ops) - 1))
                            yts.append(yt)

                        for cb in range(2):
                            va = vsl(gs0 + cb, h)
                            quad_k, klin = quads_k[cb]
                            st = (sc == 0 and cb == 0)
                            sp = (sc == NSC_B - 1 and cb == 1)
                            nc.tensor.matmul(kvq[:, 0:65], quad_k[:, 0:128],
                                             va, start=st, stop=sp)
                            nc.tensor.matmul(kvq[:, 65:130],
                                             quad_k[:, 128:256], va,
                                             start=st, stop=sp)
                            nc.tensor.matmul(kvlo[:], klin[:], va,
                                             start=st, stop=sp)
                        if sc < NSC_B - 1:
                            nc.vector.tensor_copy(kvq_sb[:], kvq[:])
                            nc.vector.tensor_copy(kvlo_sb[h17, :], kvlo[:])

                        for cb in range(2):
                            sl = slice(p0 + cb * 128, p0 + (cb + 1) * 128)
                            yt = yts[cb]
                            zi = work.tile([1, 128], F32, tag="zi")
                            nc.vector.tensor_scalar_add(zi[:], yt[64:65, :],
                                                        EPS_LIN)
                            nc.vector.reciprocal(zi[:], zi[:])
                            zb = psc.tile([64, 128], F32, tag="zb")
                            nc.tensor.matmul(zb[:], ones64_f[:], zi[:],
                                             start=True, stop=True)
                            ysb = work.tile([64, 128], BF16, tag="ysb")
                            nc.scalar.copy(ysb[:], yt[0:64, :])
                            nc.vector.tensor_tensor(catL[hd, sl],
                                                    ysb[:], zb[:],
                                                    ALU.mult)

        # ---- sliding window attention --------------------------------
        with tc.tile_pool(name="pst", bufs=2, space="PSUM") as pst, \
             tc.tile_pool(name="pyw", bufs=2, space="PSUM") as pyw, \
             tc.tile_pool(name="pzw", bufs=2, space="PSUM") as pzw:
            for b in range(B):
                for c in range(T // SUB):
                    p0 = b * T + c * SUB
                    sl = slice(p0, p0 + SUB)
                    for h in range(HPC):
                        hd = slice(h * DH, (h + 1) * DH)
                        ytw = pyw.tile([65, 128], F32, tag="ytw")
                        sblocks = [c] if c == 0 else [c - 1, c]
                        for i, sb in enumerate(sblocks):
                            ssl = slice(b * T + sb * SUB,
                                        b * T + (sb + 1) * SUB)
                            st_ps = pst.tile([128, 128], F32, tag="stps")
                            nc.tensor.matmul(st_ps[:], kT[hd, ssl],
                                             qT[hd, sl], start=True,
                                             stop=True)
                            pexp = work.tile([128, 128], BF16, tag="pexp")
                            nc.scalar.activation(pexp[:], st_ps[:], AF.Exp,
                                                 bias=0.0, scale=QK_SCALE)
                            msk = cs["mwd"] if sb == c else cs["mwp"]
                            nc.vector.tensor_tensor(pexp[:], pexp[:], msk[:],
                                                    ALU.mult)
                            nc.tensor.matmul(
                                ytw[:], vsl(b * (T // SUB) + sb, h), pexp[:],
                                start=(i == 0), stop=(i == len(sblocks) - 1))
                        ziw = work.tile([1, 128], F32, tag="ziw")
                        nc.vector.reciprocal(ziw[:], ytw[64:65, :])
                        zbw = pzw.tile([64, 128], F32, tag="zbw")
                        nc.tensor.matmul(zbw[:], ones64_f[:], ziw[:],
                                         start=True, stop=True)
                        ywsb = work.tile([64, 128], BF16, tag="ywsb")
                        nc.scalar.copy(ywsb[:], ytw[0:64, :])
                        nc.vector.tensor_tensor(catW[hd, sl], ywsb[:],
                                                zbw[:], ALU.mult)

        # ---- out-projection ------------------------------------------
        with tc.tile_pool(name="pop", bufs=2, space="PSUM") as pop, \
             tc.tile_pool(name="outp", bufs=3) as outp:
            for s in range(NSUB):
                sl = slice(s * SUB, (s + 1) * SUB)
                op = pop.tile([128, D], F32, tag="op")
                for hf in range(2):
                    c512 = slice(hf * 512, (hf + 1) * 512)
                    nc.tensor.matmul(op[:, c512], catL[:, sl],
                                     cs["w1"][:, c512], start=True, stop=False)
                    nc.tensor.matmul(op[:, c512], catW[:, sl],
                                     cs["w2"][:, c512], start=False, stop=True)
                ob = outp.tile([128, D], BF16, tag="ob")
                nc.scalar.copy(ob[:], op[:])
                nc.sync.dma_start(dram["out"][sl, :], ob[:])


_NC_CACHE = None


def _get_nc():
    global _NC_CACHE
    if _NC_CACHE is None:
        _NC_CACHE = build_bass()
    return _NC_CACHE


def _host_prep(x, norm_w, Wq, Wk, Wv, Wqf, Wkf, Wout):
    xp = np.ascontiguousarray(x.reshape(P, D).T).astype(BF)
    nw = norm_w.astype(np.float64)
    wq_f = nw[:, None] * Wq.astype(np.float64)
    wk_f = nw[:, None] * Wk.astype(np.float64)
    wv_f = nw[:, None] * Wv.astype(np.float64)

    si = np.arange(128)[:, None]
    ci = np.arange(128)[None, :]
    mtri = (si <= ci).astype(np.float32)
    mwd = ((si <= ci) & (si >= ci - WINDOW)).astype(np.float32)
    mwp = (si >= ci + WINDOW).astype(np.float32)

    wfq = np.zeros((128, 49), np.float32)
    wfq[0:64, 0:16] = Wqf
    wfq[64:128, 32:48] = Wqf
    wfk = np.zeros((128, 49), np.float32)
    wfk[0:64, 0:16] = Wkf
    wfk[64:128, 32:48] = Wkf

    in_maps = []
    for c in range(NCORES):
        csl = slice(c * 128, (c + 1) * 128)
        in_maps.append({
            "xT": xp,
            "wq": wq_f[:, csl].astype(BF),
            "wk": wk_f[:, csl].astype(BF),
            "wv": wv_f[:, csl].astype(BF),
            "wfq": wfq.astype(BF),
            "wfk": wfk.astype(BF),
            "wqf1": np.vstack([Wqf, Wqf]).astype(BF),
            "wkf1": np.vstack([Wkf, Wkf]).astype(BF),
            "w1": Wout[csl, :].astype(BF),
            "w2": Wout[1024 + c * 128:1024 + (c + 1) * 128, :].astype(BF),
            "mtri": mtri.astype(BF),
            "mwd": mwd.astype(BF),
            "mwp": mwp.astype(BF),
            "ident": np.eye(128, dtype=np.float32).astype(BF),
            "onesP": np.ones((1, P), np.float32).astype(BF),
        })
    return in_maps


def kernel(x, norm_w, Wq, Wk, Wv, Wqf, Wkf, Wout) -> np.ndarray:
    x = np.asarray(x, np.float32)
    in_maps = _host_prep(
        x, np.asarray(norm_w, np.float32), np.asarray(Wq, np.float32),
        np.asarray(Wk, np.float32), np.asarray(Wv, np.float32),
        np.asarray(Wqf, np.float32), np.asarray(Wkf, np.float32),
        np.asarray(Wout, np.float32))
    nc = _get_nc()
    res = run_bass_kernel_spmd(nc, in_maps, list(range(NCORES)))
    acc = np.zeros((P, D), np.float32)
    for c in range(NCORES):
        acc += res.results[c]["out"].astype(np.float32)
    return (x.reshape(P, D) + acc).reshape(B, T, D).astype(np.float32)


# revision 22
# speedup vs baseline: 1.0055x; 1.0055x over previous
"""BasedAttention Trainium2 kernel — nn_BasedAttention_82214263980185.

Head-sharded across 8 NeuronCores (2 heads/core): column-parallel QKV,
per-head taylor linear attention (factorized phi) + banded sliding-window
attention, row-parallel out-proj with host-side partial reduction.

Math notes:
  - reference phi(x) = [1, x, tri-scaled quad] gives
    phi(q).phi(k) = 1 + s + 0.25 s^2  (s = qf.kf).  We use the equivalent
    full-outer 256-feature quad block scaled 2^-0.25 per side plus
    [x, ones]: identical inner products, rectangular construction.
  - Intra-chunk scores: A = (1 + 0.5 s)^2 = 1 + s + 0.25 s^2 directly.
  - rmsnorm: norm_w folds into QKV weights on host; the per-row 1/rms
    factor r applies to q, k, v after projection (all linear in r).
"""

import math
import sys

for _p in ("/opt/trn_rl_repo",):
    if _p not in sys.path:
        sys.path.insert(0, _p)

import numpy as np
import ml_dtypes

import concourse.bass as bass
import concourse.mybir as mybir
import concourse.tile as tile
from concourse.bass_utils import run_bass_kernel_spmd

F32 = mybir.dt.float32
BF16 = mybir.dt.bfloat16
AF = mybir.ActivationFunctionType
ALU = mybir.AluOpType
BF = ml_dtypes.bfloat16

B, T, D = 2, 2048, 1024
P = B * T          # 4096 positions
NH, DH, FT = 16, 64, 16
HPC = 2            # heads per core
NCORES = 8
WINDOW = 64
EPS_NORM = 1e-6
EPS_LIN = 1e-6
SUB = 128          # position sub-chunk (partition tile)
NSUB = P // SUB    # 32
SC = 256           # linear-attention scan chunk
NSC_B = T // SC    # 8 scan chunks per (b,h) sequence
QK_SCALE = 1.0 / math.sqrt(DH)
QUAD_PRE = 2.0 ** (-0.25)


def _fix_tile_drain():
    """walrus here accepts only 1 sync-wait on the Tile tail drain; spread
    the global-clock waits over sequencer nop carriers."""
    if getattr(tile.TileContext, "_drain_fix", False):
        return
    from concourse.tile import ScopedClock

    def _patched(self, tick_clock, wait_clock):
        nc = self.nc
        carriers = [nc.sync.nop(nofuse=True) for _ in range(30)]
        drain_inst = nc.sync.drain()
        wait_clock.add_sem_waits(
            drain_inst.ins, ScopedClock({None: tick_clock.global_clock})
        )
        si = drain_inst.ins.sync_info
        waits = list(si.on_wait) if si is not None else []
        if len(waits) > 1:
            keep, rest = waits[:1], waits[1:]
            assert len(rest) <= len(carriers), f"too many waits: {len(waits)}"
            for c, w in zip(carriers, rest):
                c.ins.sync_info = mybir.SyncInfo(on_wait=[w], on_update=[])
            drain_inst.ins.sync_info = mybir.SyncInfo(
                on_wait=keep, on_update=list(si.on_update)
            )
        nc.all_engine_barrier()
        assert self.sems is not None
        popped = nc._tile_sem_poison_stack.pop()
        assert popped is self._sem_poison
        nc.clear_and_free_semaphores(list(self.sems.allocated().values()))
        nc.all_engine_barrier()

    tile.TileContext._drain_and_barrier = _patched
    tile.TileContext._drain_fix = True


def _split_excess_waits(nc, limit=1):
    """walrus in this container rejects instructions with more than one
    embedded sync-wait; hoist excess waits onto preceding same-engine nops."""
    n = 0
    for f in nc.m.functions:
        for b in f.blocks:
            insts = b.instructions
            out = []
            changed = False
            for ins in insts:
                si = ins.sync_info
                waits = list(si.on_wait) if si is not None else []
                if len(waits) > limit:
                    changed = True
                    for w in waits[:-limit]:
                        n += 1
                        out.append(mybir.InstNoOp(
                            name=f"waitnop-{n}", engine=ins.engine,
                            bass_nofuse=True,
                            sync_info=mybir.SyncInfo(on_wait=[w],
                                                     on_update=[])))
                    ins.sync_info = mybir.SyncInfo(
                        on_wait=waits[-limit:], on_update=list(si.on_update))
                out.append(ins)
            if changed:
                b.instructions = out
    return n


def build_bass():
    _fix_tile_drain()
    nc = bass.Bass()
    dram = {}
    for name, shape in [
        ("xT", [D, P]), ("wq", [D, 128]), ("wk", [D, 128]), ("wv", [D, 128]),
        ("wfq", [128, 49]), ("wfk", [128, 49]),
        ("wqf1", [128, FT]), ("wkf1", [128, FT]),
        ("w1", [128, D]), ("w2", [128, D]),
        ("mtri", [128, 128]), ("mwd", [128, 128]), ("mwp", [128, 128]),
        ("onesP", [1, P]),
        ("ident", [128, 128]),
    ]:
        dram[name] = nc.dram_tensor(name, shape, BF16, kind="ExternalInput")
    dram["out"] = nc.dram_tensor("out", [P, D], BF16, kind="ExternalOutput")
    dram["scr1"] = nc.dram_tensor("scr1", [P], F32)
    dram["scr2"] = nc.dram_tensor("scr2", [P], F32)
    with tile.TileContext(nc) as tc:
        _emit(nc, tc, dram)
    _split_excess_waits(nc)
    return nc


def _emit(nc, tc, dram):
    from contextlib import ExitStack

    with ExitStack() as ctx:
        const = ctx.enter_context(tc.tile_pool(name="const", bufs=1))
        big = ctx.enter_context(tc.tile_pool(name="big", bufs=1))
        work = ctx.enter_context(tc.tile_pool(name="work", bufs=3))

        # ---- constants -----------------------------------------------
        cs = {}
        for name in ("ident", "mtri", "mwd", "mwp",
                     "wfq", "wfk", "wqf1", "wkf1", "w1", "w2"):
            d = dram[name]
            t_ = const.tile(list(d.shape), BF16, tag=name)
            nc.sync.dma_start(t_[:], d[:])
            cs[name] = t_
        for name in ("wq", "wk", "wv"):
            d = dram[name]
            t_ = const.tile([128, 8 * 128], BF16, tag=name)
            for kk in range(8):
                nc.sync.dma_start(t_[:, kk * 128:(kk + 1) * 128],
                                  d[kk * 128:(kk + 1) * 128, :])
            cs[name] = t_
        ones_col_b = const.tile([128, 1], BF16, tag="ocb")
        nc.gpsimd.memset(ones_col_b[:], 1.0)
        ones64_f = const.tile([1, 64], F32, tag="o64")
        nc.gpsimd.memset(ones64_f[:], 1.0)
        ones128_f = const.tile([1, 128], F32, tag="o128")
        nc.gpsimd.memset(ones128_f[:], 1.0)
        epsn_col = const.tile([128, 1], F32, tag="epsn")
        nc.gpsimd.memset(epsn_col[:], EPS_NORM)

        # ---- big persistent tiles ------------------------------------
        qT = big.tile([128, P], BF16, tag="qT")
        kT = big.tile([128, P], BF16, tag="kT")
        Vt = big.tile([128, NSUB * 130], BF16, tag="Vt")
        qfT = big.tile([64, P], BF16, tag="qfT")   # rows 16, 48 = ones
        kfT = big.tile([64, P], BF16, tag="kfT")
        catL = big.tile([128, P], BF16, tag="catL")
        catW = big.tile([128, P], BF16, tag="catW")
        r32 = big.tile([128, NSUB], F32, tag="r32")
        r_row = big.tile([1, P], F32, tag="rrow")
        sq_row = big.tile([1, P], F32, tag="sqrow")

        def vsl(gsub, h):
            base = gsub * 130 + 65 * h
            return Vt[:, base:base + 65]

        with tc.tile_pool(name="xp", bufs=1) as xp:
            xt_sb = xp.tile([128, 8 * P], BF16, tag="xt")
            xv = [xt_sb[:, kk * P:(kk + 1) * P] for kk in range(8)]
            for kk in range(8):
                nc.sync.dma_start(xv[kk],
                                  dram["xT"][kk * 128:(kk + 1) * 128, :])

            # ---- rmsnorm scale r -------------------------------------
            with tc.tile_pool(name="psq", bufs=1, space="PSUM") as psq:
                for pc in range(8):
                    sl = slice(pc * 512, (pc + 1) * 512)
                    sq_ps = psq.tile([1, 512], F32, tag="sqps")
                    for kk in range(8):
                        sqt = work.tile([128, 512], BF16, tag="sq")
                        src = xv[kk][:, sl]
                        if kk % 2 == 0:
                            nc.scalar.activation(sqt[:], src, AF.Square)
                        else:
                            nc.vector.tensor_tensor(sqt[:], src, src, ALU.mult)
                        nc.tensor.matmul(sq_ps[:], ones_col_b[:], sqt[:],
                                         start=(kk == 0), stop=(kk == 7))
                    nc.scalar.copy(sq_row[0:1, sl], sq_ps[:])
            # (1,P) -> (128,32) via DRAM bounce: r32[o, s] = row[s*128+o]
            nc.sync.dma_start(dram["scr1"][:], sq_row[:])
            nc.sync.dma_start(
                r32[:], dram["scr1"][:].rearrange("(s o) -> o s", o=128))
            nc.scalar.activation(r32[:], r32[:], AF.Sqrt,
                                 bias=epsn_col[:], scale=1.0 / D)
            nc.vector.reciprocal(r32[:], r32[:])
            nc.sync.dma_start(
                dram["scr2"][:].rearrange("(s o) -> o s", o=128), r32[:])
            nc.sync.dma_start(r_row[:], dram["scr2"][:])

            # ---- q/k projections (d-part) ----------------------------
            wq8 = [cs["wq"][:, kk * 128:(kk + 1) * 128] for kk in range(8)]
            wk8 = [cs["wk"][:, kk * 128:(kk + 1) * 128] for kk in range(8)]
            wv8 = [cs["wv"][:, kk * 128:(kk + 1) * 128] for kk in range(8)]
            with tc.tile_pool(name="ppj", bufs=2, space="PSUM") as ppj:
                for pc in range(8):
                    sl = slice(pc * 512, (pc + 1) * 512)
                    rb_ps = ppj.tile([128, 512], F32, tag="rb")
                    nc.tensor.matmul(rb_ps[:], ones128_f[:], r_row[0:1, sl],
                                     start=True, stop=True)
                    rb_sb = work.tile([128, 512], F32, tag="rbsb")
                    nc.scalar.copy(rb_sb[:], rb_ps[:])
                    for dst, w8 in ((qT, wq8), (kT, wk8)):
                        pj = ppj.tile([128, 512], F32, tag="pj")
                        for kk in range(8):
                            nc.tensor.matmul(pj[:], w8[kk], xv[kk][:, sl],
                                             start=(kk == 0), stop=(kk == 7))
                        nc.vector.tensor_tensor(dst[:, sl], pj[:], rb_sb[:],
                                                ALU.mult)

            # ---- V (pos-part, r-scaled, ones col) --------------------
            with tc.tile_pool(name="pv", bufs=2, space="PSUM") as pv:
                for s in range(NSUB):
                    sl = slice(s * SUB, (s + 1) * SUB)
                    vp = pv.tile([128, 128], F32, tag="vp")
                    for kk in range(8):
                        nc.tensor.matmul(vp[:], xv[kk][:, sl], wv8[kk],
                                         start=(kk == 0), stop=(kk == 7))
                    rcol = r32[:, s:s + 1]
                    for h in range(HPC):
                        va = vsl(s, h)
                        nc.vector.tensor_scalar_mul(
                            va[:, 0:64], vp[:, 64 * h:64 * h + 64], rcol)
                        nc.gpsimd.memset(va[:, 64:65], 1.0)

        # ---- qfT / kfT (17-part per head, rows 16/33 ones) -----------
        with tc.tile_pool(name="pf", bufs=2, space="PSUM") as pf:
            for pc in range(8):
                sl = slice(pc * 512, (pc + 1) * 512)
                for dst, wf, src in ((qfT, cs["wfq"], qT), (kfT, cs["wfk"], kT)):
                    fp = pf.tile([49, 512], F32, tag="fp")
                    nc.tensor.matmul(fp[:], wf[:], src[:, sl],
                                     start=True, stop=True)
                    nc.vector.tensor_copy(dst[0:49, sl], fp[:])
        nc.sync.dma_start(qfT[16:17, :], dram["onesP"][:])
        nc.sync.dma_start(qfT[48:49, :], dram["onesP"][:])

        # ---- linear attention scan -----------------------------------
        with tc.tile_pool(name="pkv", bufs=1, space="PSUM") as pkv, \
             tc.tile_pool(name="psc", bufs=1, space="PSUM") as psc, \
             tc.tile_pool(name="pyt", bufs=2, space="PSUM") as pyt, \
             tc.tile_pool(name="phi", bufs=3) as phip:
            for b in range(B):
                for h in range(HPC):
                    hd = slice(h * DH, (h + 1) * DH)
                    h17 = slice(h * 32, h * 32 + 17)
                    h16 = slice(h * 32, h * 32 + 16)
                    kvq = pkv.tile([128, 130], F32, tag="kvq")
                    kvlo = pkv.tile([17, 65], F32, tag="kvlo")
                    kvq_sb = work.tile([128, 130], BF16, tag="kvqs")
                    kvlo_sb = work.tile([49, 65], BF16, tag="kvlos")
                    for sc in range(NSC_B):
                        p0 = b * T + sc * SC
                        gs0 = p0 // SUB
                        quads_q, quads_k = [], []
                        for cb in range(2):
                            sl = slice(p0 + cb * 128, p0 + (cb + 1) * 128)
                            qk_ps = psc.tile([128, 32], F32, tag="qkps")
                            nc.tensor.matmul(qk_ps[:, 0:16], qT[hd, sl],
                                             cs["wqf1"][hd, :], start=True,
                                             stop=True)
                            nc.tensor.matmul(qk_ps[:, 16:32], kT[hd, sl],
                                             cs["wkf1"][hd, :], start=True,
                                             stop=True)
                            qfp = phip.tile([128, FT], BF16, tag="qfp")
                            kfp = phip.tile([128, FT], BF16, tag="kfp")
                            klin = phip.tile([128, 17], BF16, tag="klin")
                            nc.scalar.activation(qfp[:], qk_ps[:, 0:16],
                                                 AF.Copy, bias=0.0,
                                                 scale=QUAD_PRE)
                            nc.scalar.activation(kfp[:], qk_ps[:, 16:32],
                                                 AF.Copy, bias=0.0,
                                                 scale=QUAD_PRE)
                            nc.scalar.copy(klin[:, 0:16], qk_ps[:, 16:32])
                            nc.gpsimd.memset(klin[:, 16:17], 1.0)
                            quad_q = phip.tile([128, 256], BF16, tag="qq")
                            quad_k = phip.tile([128, 256], BF16, tag="qk")
                            for qd, fsrc in ((quad_q, qfp), (quad_k, kfp)):
                                g1 = fsrc[:].unsqueeze(2).broadcast_to(
                                    (128, FT, FT))
                                g2 = fsrc[:].unsqueeze(1).broadcast_to(
                                    (128, FT, FT))
                                nc.vector.tensor_tensor(
                                    qd[:].rearrange("p (i j) -> p i j", i=FT),
                                    g1, g2, ALU.mult)
                            q1sb = phip.tile([128, 128], BF16, tag="q1sb")
                            q2sb = phip.tile([128, 128], BF16, tag="q2sb")
                            for half, qsb in ((0, q1sb), (1, q2sb)):
                                nc.sync.dma_start_transpose(
                                    qsb[:],
                                    quad_q[:, half * 128:(half + 1) * 128])
                            quads_q.append((q1sb, q2sb))
                            quads_k.append((quad_k, klin))

                        yts = []
                        for cb in range(2):
                            sl = slice(p0 + cb * 128, p0 + (cb + 1) * 128)
                            yt = pyt.tile([65, 128], F32, tag="yt")
                            ops = []
                            for sb in range(cb + 1):
                                ssl = slice(p0 + sb * 128,
                                            p0 + (sb + 1) * 128)
                                s_ps = psc.tile([128, 128], F32, tag="sps")
                                nc.tensor.matmul(s_ps[:], kfT[h16, ssl],
                                                 qfT[h16, sl],
                                                 start=True, stop=True)
                                a_sb = work.tile([128, 128], BF16, tag="asb")
                                nc.scalar.activation(a_sb[:], s_ps[:],
                                                     AF.Square,
                                                     bias=1.0, scale=0.5)
                                if sb == cb:
                                    nc.vector.tensor_tensor(
                                        a_sb[:], a_sb[:], cs["mtri"][:],
                                        ALU.mult)
                                ops.append((vsl(gs0 + sb, h), a_sb[:]))
                            if sc > 0:
                                q1sb, q2sb = quads_q[cb]
                                ops.append((kvq_sb[:, 0:65], q1sb[:]))
                                ops.append((kvq_sb[:, 65:130], q2sb[:]))
                                ops.append((kvlo_sb[h17, :], qfT[h17, sl]))
                            for i, (lt, rt) in enumerate(ops):
                                nc.tensor.matmul(yt[:], lt, rt,
                                                 start=(i == 0),
                                                 stop=(i == len(ops) - 1))
                            yts.append(yt)

                        for cb in range(2):
                            va = vsl(gs0 + cb, h)
                            quad_k, klin = quads_k[cb]
                            st = (sc == 0 and cb == 0)
                            sp = (sc == NSC_B - 1 and cb == 1)
                            nc.tensor.matmul(kvq[:, 0:65], quad_k[:, 0:128],
                                             va, start=st, stop=sp)
                            nc.tensor.matmul(kvq[:, 65:130],
                                             quad_k[:, 128:256], va,
                                             start=st, stop=sp)
                            nc.tensor.matmul(kvlo[:], klin[:], va,
                                             start=st, stop=sp)
                        if sc < NSC_B - 1:
                            nc.vector.tensor_copy(kvq_sb[:], kvq[:])
                            nc.vector.tensor_copy(kvlo_sb[h17, :], kvlo[:])

                        for cb in range(2):
                            sl = slice(p0 + cb * 128, p0 + (cb + 1) * 128)
                            yt = yts[cb]
                            zi = work.tile([1, 128], F32, tag="zi")
                            nc.vector.reciprocal(zi[:], yt[64:65, :])
                            zb = psc.tile([64, 128], F32, tag="zb")
                            nc.tensor.matmul(zb[:], ones64_f[:], zi[:],
                                             start=True, stop=True)
                            ysb = work.tile([64, 128], BF16, tag="ysb")
                            nc.scalar.copy(ysb[:], yt[0:64, :])
                            nc.vector.tensor_tensor(catL[hd, sl],
                                                    ysb[:], zb[:],
                                                    ALU.mult)

        # ---- sliding window attention --------------------------------
        with tc.tile_pool(name="pst", bufs=2, space="PSUM") as pst, \
             tc.tile_pool(name="pyw", bufs=2, space="PSUM") as pyw, \
             tc.tile_pool(name="pzw", bufs=2, space="PSUM") as pzw:
            for b in range(B):
                for c in range(T // SUB):
                    p0 = b * T + c * SUB
                    sl = slice(p0, p0 + SUB)
                    for h in range(HPC):
                        hd = slice(h * DH, (h + 1) * DH)
                        ytw = pyw.tile([65, 128], F32, tag="ytw")
                        sblocks = [c] if c == 0 else [c - 1, c]
                        for i, sb in enumerate(sblocks):
                            ssl = slice(b * T + sb * SUB,
                                        b * T + (sb + 1) * SUB)
                            st_ps = pst.tile([128, 128], F32, tag="stps")
                            nc.tensor.matmul(st_ps[:], kT[hd, ssl],
                                             qT[hd, sl], start=True,
                                             stop=True)
                            pexp = work.tile([128, 128], BF16, tag="pexp")
                            nc.scalar.activation(pexp[:], st_ps[:], AF.Exp,
                                                 bias=0.0, scale=QK_SCALE)
                            msk = cs["mwd"] if sb == c else cs["mwp"]
                            nc.vector.tensor_tensor(pexp[:], pexp[:], msk[:],
                                                    ALU.mult)
                            nc.tensor.matmul(
                                ytw[:], vsl(b * (T // SUB) + sb, h), pexp[:],
                                start=(i == 0), stop=(i == len(sblocks) - 1))
                        ziw = work.tile([1, 128], F32, tag="ziw")
                        nc.vector.reciprocal(ziw[:], ytw[64:65, :])
                        zbw = pzw.tile([64, 128], F32, tag="zbw")
                        nc.tensor.matmul(zbw[:], ones64_f[:], ziw[:],
                                         start=True, stop=True)
                        ywsb = work.tile([64, 128], BF16, tag="ywsb")
                        nc.scalar.copy(ywsb[:], ytw[0:64, :])
                        nc.vector.tensor_tensor(catW[hd, sl], ywsb[:],
                                                zbw[:], ALU.mult)

        # ---- out-projection ------------------------------------------
        with tc.tile_pool(name="pop", bufs=2, space="PSUM") as pop, \
             tc.tile_pool(name="outp", bufs=3) as outp:
            for s in range(NSUB):
                sl = slice(s * SUB, (s + 1) * SUB)
                op = pop.tile([128, D], F32, tag="op")
                for hf in range(2):
                    c512 = slice(hf * 512, (hf + 1) * 512)
                    nc.tensor.matmul(op[:, c512], catL[:, sl],
                                     cs["w1"][:, c512], start=True, stop=False)
                    nc.tensor.matmul(op[:, c512], catW[:, sl],
                                     cs["w2"][:, c512], start=False, stop=True)
                ob = outp.tile([128, D], BF16, tag="ob")
                nc.scalar.copy(ob[:], op[:])
                nc.sync.dma_start(dram["out"][sl, :], ob[:])


_NC_CACHE = None


def _get_nc():
    global _NC_CACHE
    if _NC_CACHE is None:
        _NC_CACHE = build_bass()
    return _NC_CACHE


def _host_prep(x, norm_w, Wq, Wk, Wv, Wqf, Wkf, Wout):
    xp = np.ascontiguousarray(x.reshape(P, D).T).astype(BF)
    nw = norm_w.astype(np.float64)
    wq_f = nw[:, None] * Wq.astype(np.float64)
    wk_f = nw[:, None] * Wk.astype(np.float64)
    wv_f = nw[:, None] * Wv.astype(np.float64)

    si = np.arange(128)[:, None]
    ci = np.arange(128)[None, :]
    mtri = (si <= ci).astype(np.float32)
    mwd = ((si <= ci) & (si >= ci - WINDOW)).astype(np.float32)
    mwp = (si >= ci + WINDOW).astype(np.float32)

    wfq = np.zeros((128, 49), np.float32)
    wfq[0:64, 0:16] = Wqf
    wfq[64:128, 32:48] = Wqf
    wfk = np.zeros((128, 49), np.float32)
    wfk[0:64, 0:16] = Wkf
    wfk[64:128, 32:48] = Wkf

    in_maps = []
    for c in range(NCORES):
        csl = slice(c * 128, (c + 1) * 128)
        in_maps.append({
            "xT": xp,
            "wq": wq_f[:, csl].astype(BF),
            "wk": wk_f[:, csl].astype(BF),
            "wv": wv_f[:, csl].astype(BF),
            "wfq": wfq.astype(BF),
            "wfk": wfk.astype(BF),
            "wqf1": np.vstack([Wqf, Wqf]).astype(BF),
            "wkf1": np.vstack([Wkf, Wkf]).astype(BF),
            "w1": Wout[csl, :].astype(BF),
            "w2": Wout[1024 + c * 128:1024 + (c + 1) * 128, :].astype(BF),
            "mtri": mtri.astype(BF),
            "mwd": mwd.astype(BF),
            "mwp": mwp.astype(BF),
            "ident": np.eye(128, dtype=np.float32).astype(BF),
            "onesP": np.ones((1, P), np.float32).astype(BF),
        })
    return in_maps


def kernel(x, norm_w, Wq, Wk, Wv, Wqf, Wkf, Wout) -> np.ndarray:
    x = np.asarray(x, np.float32)
    in_maps = _host_prep(
        x, np.asarray(norm_w, np.float32), np.asarray(Wq, np.float32),
        np.asarray(Wk, np.float32), np.asarray(Wv, np.float32),
        np.asarray(Wqf, np.float32), np.asarray(Wkf, np.float32),
        np.asarray(Wout, np.float32))
    nc = _get_nc()
    res = run_bass_kernel_spmd(nc, in_maps, list(range(NCORES)))
    acc = np.zeros((P, D), np.float32)
    for c in range(NCORES):
        acc += res.results[c]["out"].astype(np.float32)
    return (x.reshape(P, D) + acc).reshape(B, T, D).astype(np.float32)


# revision 38
# speedup vs baseline: 1.1193x; 1.1133x over previous
"""BasedAttention Trainium2 kernel — nn_BasedAttention_82214263980185.

Head-sharded across 8 NeuronCores (2 heads/core): column-parallel QKV,
per-head taylor linear attention (factorized phi) + banded sliding-window
attention, row-parallel out-proj with host-side partial reduction.

Math notes:
  - reference phi(x) = [1, x, tri-scaled quad] gives
    phi(q).phi(k) = 1 + s + 0.25 s^2  (s = qf.kf).  We use the equivalent
    full-outer 256-feature quad block scaled 2^-0.25 per side plus
    [x, ones]: identical inner products, rectangular construction.
  - Intra-chunk scores: A = (1 + 0.5 s)^2 = 1 + s + 0.25 s^2 directly.
  - rmsnorm: norm_w folds into QKV weights on host; the per-row 1/rms
    factor r applies to q, k, v after projection (all linear in r).
"""

import math
import sys

for _p in ("/opt/trn_rl_repo",):
    if _p not in sys.path:
        sys.path.insert(0, _p)

import numpy as np
import ml_dtypes

import concourse.bass as bass
import concourse.mybir as mybir
import concourse.tile as tile
from concourse.bass_utils import run_bass_kernel_spmd

F32 = mybir.dt.float32
BF16 = mybir.dt.bfloat16
AF = mybir.ActivationFunctionType
ALU = mybir.AluOpType
BF = ml_dtypes.bfloat16

B, T, D = 2, 2048, 1024
P = B * T          # 4096 positions
NH, DH, FT = 16, 64, 16
HPC = 2            # heads per core
NCORES = 8
WINDOW = 64
EPS_NORM = 1e-6
EPS_LIN = 1e-6
SUB = 128          # position sub-chunk (partition tile)
NSUB = P // SUB    # 32
SC = 256           # linear-attention scan chunk
NSC_B = T // SC    # 8 scan chunks per (b,h) sequence
QK_SCALE = 1.0 / math.sqrt(DH)
QUAD_PRE = 2.0 ** (-0.5)


def _fix_tile_drain():
    """walrus here accepts only 1 sync-wait on the Tile tail drain; spread
    the global-clock waits over sequencer nop carriers."""
    if getattr(tile.TileContext, "_drain_fix", False):
        return
    from concourse.tile import ScopedClock

    def _patched(self, tick_clock, wait_clock):
        nc = self.nc
        carriers = [nc.sync.nop(nofuse=True) for _ in range(30)]
        drain_inst = nc.sync.drain()
        wait_clock.add_sem_waits(
            drain_inst.ins, ScopedClock({None: tick_clock.global_clock})
        )
        si = drain_inst.ins.sync_info
        waits = list(si.on_wait) if si is not None else []
        if len(waits) > 1:
            keep, rest = waits[:1], waits[1:]
            assert len(rest) <= len(carriers), f"too many waits: {len(waits)}"
            for c, w in zip(carriers, rest):
                c.ins.sync_info = mybir.SyncInfo(on_wait=[w], on_update=[])
            drain_inst.ins.sync_info = mybir.SyncInfo(
                on_wait=keep, on_update=list(si.on_update)
            )
        nc.all_engine_barrier()
        assert self.sems is not None
        popped = nc._tile_sem_poison_stack.pop()
        assert popped is self._sem_poison
        nc.clear_and_free_semaphores(list(self.sems.allocated().values()))
        nc.all_engine_barrier()

    tile.TileContext._drain_and_barrier = _patched
    tile.TileContext._drain_fix = True


def _split_excess_waits(nc, limit=1):
    """walrus in this container rejects instructions with more than one
    embedded sync-wait; hoist excess waits onto preceding same-engine nops."""
    n = 0
    for f in nc.m.functions:
        for b in f.blocks:
            insts = b.instructions
            out = []
            changed = False
            for ins in insts:
                si = ins.sync_info
                waits = list(si.on_wait) if si is not None else []
                if len(waits) > limit:
                    changed = True
                    for w in waits[:-limit]:
                        n += 1
                        out.append(mybir.InstNoOp(
                            name=f"waitnop-{n}", engine=ins.engine,
                            bass_nofuse=True,
                            sync_info=mybir.SyncInfo(on_wait=[w],
                                                     on_update=[])))
                    ins.sync_info = mybir.SyncInfo(
                        on_wait=waits[-limit:], on_update=list(si.on_update))
                out.append(ins)
            if changed:
                b.instructions = out
    return n


def build_bass():
    _fix_tile_drain()
    nc = bass.Bass()
    dram = {}
    for name, shape in [
        ("xT", [D, P]), ("wq", [D, 128]), ("wk", [D, 128]), ("wv", [D, 128]),
        ("wfq", [128, 49]), ("wfk", [128, 49]),
        ("wqf1", [128, FT]), ("wkf1", [128, FT]),
        ("w1", [128, D]), ("w2", [128, D]),
        ("mtri", [128, 128]), ("mwd", [128, 128]), ("mwp", [128, 128]),
        ("onesP", [1, P]),
        ("ident", [128, 128]),
    ]:
        dram[name] = nc.dram_tensor(name, shape, BF16, kind="ExternalInput")
    dram["out"] = nc.dram_tensor("out", [P, D], BF16, kind="ExternalOutput")
    dram["scr1"] = nc.dram_tensor("scr1", [P], F32)
    dram["scr2"] = nc.dram_tensor("scr2", [P], F32)
    with tile.TileContext(nc) as tc:
        _emit(nc, tc, dram)
    _split_excess_waits(nc)
    return nc


def _emit(nc, tc, dram):
    from contextlib import ExitStack

    with ExitStack() as ctx:
        const = ctx.enter_context(tc.tile_pool(name="const", bufs=1))
        big = ctx.enter_context(tc.tile_pool(name="big", bufs=1))
        work = ctx.enter_context(tc.tile_pool(name="work", bufs=3))

        # ---- constants -----------------------------------------------
        cs = {}
        for name in ("ident", "mtri", "mwd", "mwp",
                     "wfq", "wfk", "wqf1", "wkf1", "w1", "w2"):
            d = dram[name]
            t_ = const.tile(list(d.shape), BF16, tag=name)
            nc.sync.dma_start(t_[:], d[:])
            cs[name] = t_
        for name in ("wq", "wk", "wv"):
            d = dram[name]
            t_ = const.tile([128, 8 * 128], BF16, tag=name)
            for kk in range(8):
                nc.sync.dma_start(t_[:, kk * 128:(kk + 1) * 128],
                                  d[kk * 128:(kk + 1) * 128, :])
            cs[name] = t_
        ones_col_b = const.tile([128, 1], BF16, tag="ocb")
        nc.gpsimd.memset(ones_col_b[:], 1.0)
        ones64_f = const.tile([1, 64], F32, tag="o64")
        nc.gpsimd.memset(ones64_f[:], 1.0)
        ones128_f = const.tile([1, 128], F32, tag="o128")
        nc.gpsimd.memset(ones128_f[:], 1.0)
        epsn_col = const.tile([128, 1], F32, tag="epsn")
        nc.gpsimd.memset(epsn_col[:], EPS_NORM)

        # ---- big persistent tiles ------------------------------------
        qT = big.tile([128, P], BF16, tag="qT")
        kT = big.tile([128, P], BF16, tag="kT")
        Vt = big.tile([128, NSUB * 130], BF16, tag="Vt")
        vT = big.tile([128, P], BF16, tag="vT")
        qfT = big.tile([64, P], BF16, tag="qfT")   # rows 16, 48 = ones
        kfT = big.tile([64, P], BF16, tag="kfT")
        catL = big.tile([128, P], BF16, tag="catL")
        catW = big.tile([128, P], BF16, tag="catW")
        r32 = big.tile([128, NSUB], F32, tag="r32")
        r_row = big.tile([1, P], F32, tag="rrow")
        sq_row = big.tile([1, P], F32, tag="sqrow")

        def vsl(gsub, h):
            # per sub: [v_h0 (64) | 1 | v_h1 (64) | 1]
            base = gsub * 130 + 65 * h
            return Vt[:, base:base + 65]

        def yrow(h):
            return slice(0, 64), slice(64, 65)

        with tc.tile_pool(name="xp", bufs=1) as xp:
            xt_sb = xp.tile([128, 8 * P], BF16, tag="xt")
            xv = [xt_sb[:, kk * P:(kk + 1) * P] for kk in range(8)]
            for qq in range(4):
                csl = slice(qq * (P // 4), (qq + 1) * (P // 4))
                for kk in range(8):
                    nc.sync.dma_start(xv[kk][:, csl],
                                      dram["xT"][kk * 128:(kk + 1) * 128,
                                                 csl])

            # ---- rmsnorm scale r -------------------------------------
            with tc.tile_pool(name="psq", bufs=1, space="PSUM") as psq:
                for pc in range(8):
                    sl = slice(pc * 512, (pc + 1) * 512)
                    sq_ps = psq.tile([1, 512], F32, tag="sqps")
                    for kk in range(8):
                        sqt = work.tile([128, 512], BF16, tag="sq")
                        src = xv[kk][:, sl]
                        if kk % 2 == 0:
                            nc.scalar.activation(sqt[:], src, AF.Square)
                        else:
                            nc.vector.tensor_tensor(sqt[:], src, src, ALU.mult)
                        nc.tensor.matmul(sq_ps[:], ones_col_b[:], sqt[:],
                                         start=(kk == 0), stop=(kk == 7))
                    nc.scalar.copy(sq_row[0:1, sl], sq_ps[:])
            # (1,P) -> (128,32) via DRAM bounce: r32[o, s] = row[s*128+o]
            nc.sync.dma_start(dram["scr1"][:], sq_row[:])
            nc.sync.dma_start(
                r32[:], dram["scr1"][:].rearrange("(s o) -> o s", o=128))
            nc.scalar.activation(r32[:], r32[:], AF.Sqrt,
                                 bias=epsn_col[:], scale=1.0 / D)
            nc.vector.reciprocal(r32[:], r32[:])
            nc.sync.dma_start(
                dram["scr2"][:].rearrange("(s o) -> o s", o=128), r32[:])
            nc.sync.dma_start(r_row[:], dram["scr2"][:])

            # ---- q/k projections (d-part) ----------------------------
            wq8 = [cs["wq"][:, kk * 128:(kk + 1) * 128] for kk in range(8)]
            wk8 = [cs["wk"][:, kk * 128:(kk + 1) * 128] for kk in range(8)]
            wv8 = [cs["wv"][:, kk * 128:(kk + 1) * 128] for kk in range(8)]
            with tc.tile_pool(name="ppj", bufs=2, space="PSUM") as ppj:
                for pc in range(8):
                    sl = slice(pc * 512, (pc + 1) * 512)
                    rb_ps = ppj.tile([128, 512], F32, tag="rb")
                    nc.tensor.matmul(rb_ps[:], ones128_f[:], r_row[0:1, sl],
                                     start=True, stop=True)
                    rb_sb = work.tile([128, 512], F32, tag="rbsb")
                    nc.scalar.copy(rb_sb[:], rb_ps[:])
                    for dst, w8 in ((qT, wq8), (kT, wk8), (vT, wv8)):
                        pj = ppj.tile([128, 512], F32, tag="pj")
                        for kk in range(8):
                            nc.tensor.matmul(pj[:], w8[kk], xv[kk][:, sl],
                                             start=(kk == 0), stop=(kk == 7))
                        nc.vector.tensor_tensor(dst[:, sl], pj[:], rb_sb[:],
                                                ALU.mult)

            # ---- V pos-part via DMA transpose of vT ------------------
            for s in range(NSUB):
                sl = slice(s * SUB, (s + 1) * SUB)
                for h in range(HPC):
                    nc.sync.dma_start_transpose(
                        Vt[:, s * 130 + h * 65:s * 130 + h * 65 + 64],
                        vT[h * DH:(h + 1) * DH, sl])
                    nc.gpsimd.memset(
                        Vt[:, s * 130 + h * 65 + 64:s * 130 + h * 65 + 65],
                        1.0)

        # ---- qfT / kfT (17-part per head, rows 16/33 ones) -----------
        with tc.tile_pool(name="pf", bufs=2, space="PSUM") as pf:
            for pc in range(8):
                sl = slice(pc * 512, (pc + 1) * 512)
                for dst, wf, src in ((qfT, cs["wfq"], qT), (kfT, cs["wfk"], kT)):
                    fp = pf.tile([49, 512], F32, tag="fp")
                    nc.tensor.matmul(fp[:], wf[:], src[:, sl],
                                     start=True, stop=True)
                    nc.vector.tensor_copy(dst[0:49, sl], fp[:])
        nc.sync.dma_start(qfT[16:17, :], dram["onesP"][:])
        nc.sync.dma_start(qfT[48:49, :], dram["onesP"][:])

        # ---- linear attention scan -----------------------------------
        with tc.tile_pool(name="pkv", bufs=1, space="PSUM") as pkv, \
             tc.tile_pool(name="psc", bufs=1, space="PSUM") as psc, \
             tc.tile_pool(name="psp", bufs=2, space="PSUM") as psp2, \
             tc.tile_pool(name="pyt", bufs=2, space="PSUM") as pyt, \
             tc.tile_pool(name="phi", bufs=17) as phip:
            for b in range(B):
                for h in range(HPC):
                    hd = slice(h * DH, (h + 1) * DH)
                    h17 = slice(h * 32, h * 32 + 17)
                    h16 = slice(h * 32, h * 32 + 16)
                    kvq = pkv.tile([128, 130], F32, tag="kvq")
                    kvlo = pkv.tile([17, 65], F32, tag="kvlo")
                    kvq_sb = work.tile([128, 130], BF16, tag="kvqs")
                    kvlo_sb = work.tile([49, 65], BF16, tag="kvlos")
                    all_q, all_k = [], []
                    for sc in range(NSC_B):
                        p0 = b * T + sc * SC
                        quads_q, quads_k = [], []
                        all_q.append(quads_q)
                        all_k.append(quads_k)
                        for cb in range(2):
                            sl = slice(p0 + cb * 128, p0 + (cb + 1) * 128)
                            qk_ps = psc.tile([128, 32], F32, tag="qkps")
                            nc.tensor.matmul(qk_ps[:, 0:16], qT[hd, sl],
                                             cs["wqf1"][hd, :], start=True,
                                             stop=True)
                            nc.tensor.matmul(qk_ps[:, 16:32], kT[hd, sl],
                                             cs["wkf1"][hd, :], start=True,
                                             stop=True)
                            qfp = phip.tile([128, FT], BF16, tag="qfp")
                            klin = phip.tile([128, 17], BF16, tag="klin")
                            # host folds 2^+0.5 into wfq, 2^-0.5 into wfk:
                            # klin doubles as the quad-scaled kf.
                            nc.scalar.activation(qfp[:], qk_ps[:, 0:16],
                                                 AF.Copy, bias=0.0,
                                                 scale=0.5)
                            nc.scalar.copy(klin[:, 0:16], qk_ps[:, 16:32])
                            nc.gpsimd.memset(klin[:, 16:17], 1.0)
                            quad_q = phip.tile([128, 256], BF16, tag="qq")
                            quad_k = phip.tile([128, 256], BF16, tag="qk")
                            for qd, fsrc in ((quad_q, qfp[:]),
                                             (quad_k, klin[:, 0:16])):
                                g1 = fsrc.unsqueeze(2).broadcast_to(
                                    (128, FT, FT))
                                g2 = fsrc.unsqueeze(1).broadcast_to(
                                    (128, FT, FT))
                                nc.gpsimd.tensor_tensor(
                                    qd[:].rearrange("p (i j) -> p i j", i=FT),
                                    g1, g2, ALU.mult)
                            q1sb = phip.tile([128, 128], BF16, tag="q1sb")
                            q2sb = phip.tile([128, 128], BF16, tag="q2sb")
                            for half, qsb in ((0, q1sb), (1, q2sb)):
                                nc.sync.dma_start_transpose(
                                    qsb[:],
                                    quad_q[:, half * 128:(half + 1) * 128])
                            quads_q.append((q1sb, q2sb))
                            quads_k.append((quad_k, klin))

                    for sc in range(NSC_B):
                        p0 = b * T + sc * SC
                        gs0 = p0 // SUB
                        quads_q = all_q[sc]
                        quads_k = all_k[sc]
                        yts = []
                        for cb in range(2):
                            sl = slice(p0 + cb * 128, p0 + (cb + 1) * 128)
                            yt = pyt.tile([65, 128], F32, tag="yt")
                            ops = []
                            for sb in range(cb + 1):
                                ssl = slice(p0 + sb * 128,
                                            p0 + (sb + 1) * 128)
                                s_ps = psp2.tile([128, 128], F32, tag="sps")
                                nc.tensor.matmul(s_ps[:], kfT[h16, ssl],
                                                 qfT[h16, sl],
                                                 start=True, stop=True)
                                a_sb = work.tile([128, 128], BF16, tag="asb")
                                nc.scalar.activation(a_sb[:], s_ps[:],
                                                     AF.Square,
                                                     bias=1.0, scale=0.5)
                                if sb == cb:
                                    nc.vector.tensor_tensor(
                                        a_sb[:], a_sb[:], cs["mtri"][:],
                                        ALU.mult)
                                ops.append((vsl(gs0 + sb, h), a_sb[:]))
                            if sc > 0:
                                q1sb, q2sb = quads_q[cb]
                                ops.append((kvq_sb[:, 0:65], q1sb[:]))
                                ops.append((kvq_sb[:, 65:130], q2sb[:]))
                                ops.append((kvlo_sb[h17, :], qfT[h17, sl]))
                            for i, (lt, rt) in enumerate(ops):
                                nc.tensor.matmul(yt[:], lt, rt,
                                                 start=(i == 0),
                                                 stop=(i == len(ops) - 1))
                            yts.append(yt)

                        for cb in range(2):
                            va = vsl(gs0 + cb, h)
                            quad_k, klin = quads_k[cb]
                            st = (sc == 0 and cb == 0)
                            sp = (sc == NSC_B - 1 and cb == 1)
                            nc.tensor.matmul(kvq[:, 0:65], quad_k[:, 0:128],
                                             va, start=st, stop=sp)
                            nc.tensor.matmul(kvq[:, 65:130],
                                             quad_k[:, 128:256], va,
                                             start=st, stop=sp)
                            nc.tensor.matmul(kvlo[:], klin[:], va,
                                             start=st, stop=sp)
                        if sc < NSC_B - 1:
                            nc.vector.tensor_copy(kvq_sb[:], kvq[:])
                            nc.vector.tensor_copy(kvlo_sb[h17, :], kvlo[:])

                        sl2 = slice(p0, p0 + SC)
                        ysl, zsl = yrow(h)
                        zi = work.tile([1, 256], F32, tag="zi")
                        nc.vector.reciprocal(zi[0:1, 0:128],
                                             yts[0][zsl, :])
                        nc.vector.reciprocal(zi[0:1, 128:256],
                                             yts[1][zsl, :])
                        zb = psc.tile([64, 256], F32, tag="zb")
                        nc.tensor.matmul(zb[:], ones64_f[:], zi[:],
                                         start=True, stop=True)
                        ysb = work.tile([64, 256], BF16, tag="ysb")
                        nc.scalar.copy(ysb[:, 0:128], yts[0][ysl, :])
                        nc.scalar.copy(ysb[:, 128:256], yts[1][ysl, :])
                        nc.vector.tensor_tensor(catL[hd, sl2], ysb[:], zb[:],
                                                ALU.mult)

        # ---- sliding window attention --------------------------------
        with tc.tile_pool(name="pst", bufs=2, space="PSUM") as pst, \
             tc.tile_pool(name="pyw", bufs=2, space="PSUM") as pyw, \
             tc.tile_pool(name="pzw", bufs=2, space="PSUM") as pzw:
            for b in range(B):
                for c in range(T // SUB):
                    p0 = b * T + c * SUB
                    sl = slice(p0, p0 + SUB)
                    for h in range(HPC):
                        hd = slice(h * DH, (h + 1) * DH)
                        ytw = pyw.tile([65, 128], F32, tag="ytw")
                        sblocks = [c] if c == 0 else [c - 1, c]
                        for i, sb in enumerate(sblocks):
                            ssl = slice(b * T + sb * SUB,
                                        b * T + (sb + 1) * SUB)
                            st_ps = pst.tile([128, 128], F32, tag="stps")
                            nc.tensor.matmul(st_ps[:], kT[hd, ssl],
                                             qT[hd, sl], start=True,
                                             stop=True)
                            pexp = work.tile([128, 128], BF16, tag="pexp")
                            nc.scalar.activation(pexp[:], st_ps[:], AF.Exp,
                                                 bias=0.0, scale=QK_SCALE)
                            msk = cs["mwd"] if sb == c else cs["mwp"]
                            nc.vector.tensor_tensor(pexp[:], pexp[:], msk[:],
                                                    ALU.mult)
                            nc.tensor.matmul(
                                ytw[:], vsl(b * (T // SUB) + sb, h), pexp[:],
                                start=(i == 0),
                                stop=(i == len(sblocks) - 1))
                        ziw = work.tile([1, 128], F32, tag="ziw")
                        nc.vector.reciprocal(ziw[:], ytw[64:65, :])
                        zbw = pzw.tile([64, 128], F32, tag="zbw")
                        nc.tensor.matmul(zbw[:], ones64_f[:], ziw[:],
                                         start=True, stop=True)
                        ywsb = work.tile([64, 128], BF16, tag="ywsb")
                        nc.scalar.copy(ywsb[:], ytw[0:64, :])
                        nc.vector.tensor_tensor(catW[hd, sl], ywsb[:],
                                                zbw[:], ALU.mult)

        # ---- out-projection ------------------------------------------
        with tc.tile_pool(name="pop", bufs=2, space="PSUM") as pop, \
             tc.tile_pool(name="outp", bufs=3) as outp:
            for s in range(NSUB):
                sl = slice(s * SUB, (s + 1) * SUB)
                op = pop.tile([128, D], F32, tag="op")
                for hf in range(2):
                    c512 = slice(hf * 512, (hf + 1) * 512)
                    nc.tensor.matmul(op[:, c512], catL[:, sl],
                                     cs["w1"][:, c512], start=True, stop=False)
                    nc.tensor.matmul(op[:, c512], catW[:, sl],
                                     cs["w2"][:, c512], start=False, stop=True)
                ob = outp.tile([128, D], BF16, tag="ob")
                nc.scalar.copy(ob[:], op[:])
                nc.gpsimd.dma_start(dram["out"][sl, :], ob[:])


_NC_CACHE = None


def _get_nc():
    global _NC_CACHE
    if _NC_CACHE is None:
        _NC_CACHE = build_bass()
    return _NC_CACHE


def _host_prep(x, norm_w, Wq, Wk, Wv, Wqf, Wkf, Wout):
    xp = np.ascontiguousarray(x.reshape(P, D).T).astype(BF)
    nw = norm_w.astype(np.float64)
    wq_f = nw[:, None] * Wq.astype(np.float64)
    wk_f = nw[:, None] * Wk.astype(np.float64)
    wv_f = nw[:, None] * Wv.astype(np.float64)

    si = np.arange(128)[:, None]
    ci = np.arange(128)[None, :]
    mtri = (si <= ci).astype(np.float32)
    mwd = ((si <= ci) & (si >= ci - WINDOW)).astype(np.float32)
    mwp = (si >= ci + WINDOW).astype(np.float32)

    sq2 = math.sqrt(2.0)
    wfq = np.zeros((128, 49), np.float32)
    wfq[0:64, 0:16] = Wqf * sq2
    wfq[64:128, 32:48] = Wqf * sq2
    wfk = np.zeros((128, 49), np.float32)
    wfk[0:64, 0:16] = Wkf / sq2
    wfk[64:128, 32:48] = Wkf / sq2

    in_maps = []
    for c in range(NCORES):
        csl = slice(c * 128, (c + 1) * 128)
        in_maps.append({
            "xT": xp,
            "wq": wq_f[:, csl].astype(BF),
            "wk": wk_f[:, csl].astype(BF),
            "wv": wv_f[:, csl].astype(BF),
            "wfq": wfq.astype(BF),
            "wfk": wfk.astype(BF),
            "wqf1": (np.vstack([Wqf, Wqf]) * sq2).astype(BF),
            "wkf1": (np.vstack([Wkf, Wkf]) / sq2).astype(BF),
            "w1": Wout[csl, :].astype(BF),
            "w2": Wout[1024 + c * 128:1024 + (c + 1) * 128, :].astype(BF),
            "mtri": mtri.astype(BF),
            "mwd": mwd.astype(BF),
            "mwp": mwp.astype(BF),
            "ident": np.eye(128, dtype=np.float32).astype(BF),
            "onesP": np.ones((1, P), np.float32).astype(BF),
        })
    return in_maps


def kernel(x, norm_w, Wq, Wk, Wv, Wqf, Wkf, Wout) -> np.ndarray:
    x = np.asarray(x, np.float32)
    in_maps = _host_prep(
        x, np.asarray(norm_w, np.float32), np.asarray(Wq, np.float32),
        np.asarray(Wk, np.float32), np.asarray(Wv, np.float32),
        np.asarray(Wqf, np.float32), np.asarray(Wkf, np.float32),
        np.asarray(Wout, np.float32))
    nc = _get_nc()
    res = run_bass_kernel_spmd(nc, in_maps, list(range(NCORES)))
    acc = np.zeros((P, D), np.float32)
    for c in range(NCORES):
        acc += res.results[c]["out"].astype(np.float32)
    return (x.reshape(P, D) + acc).reshape(B, T, D).astype(np.float32)


# revision 40
# speedup vs baseline: 1.1549x; 1.0318x over previous
"""BasedAttention Trainium2 kernel — nn_BasedAttention_82214263980185.

Head-sharded across 8 NeuronCores (2 heads/core): column-parallel QKV,
per-head taylor linear attention (factorized phi) + banded sliding-window
attention, row-parallel out-proj with host-side partial reduction.

Math notes:
  - reference phi(x) = [1, x, tri-scaled quad] gives
    phi(q).phi(k) = 1 + s + 0.25 s^2  (s = qf.kf).  We use the equivalent
    full-outer 256-feature quad block scaled 2^-0.25 per side plus
    [x, ones]: identical inner products, rectangular construction.
  - Intra-chunk scores: A = (1 + 0.5 s)^2 = 1 + s + 0.25 s^2 directly.
  - rmsnorm: norm_w folds into QKV weights on host; the per-row 1/rms
    factor r applies to q, k, v after projection (all linear in r).
"""

import math
import sys

for _p in ("/opt/trn_rl_repo",):
    if _p not in sys.path:
        sys.path.insert(0, _p)

import numpy as np
import ml_dtypes

import concourse.bass as bass
import concourse.mybir as mybir
import concourse.tile as tile
from concourse.bass_utils import run_bass_kernel_spmd

F32 = mybir.dt.float32
BF16 = mybir.dt.bfloat16
AF = mybir.ActivationFunctionType
ALU = mybir.AluOpType
BF = ml_dtypes.bfloat16

B, T, D = 2, 2048, 1024
P = B * T          # 4096 positions
NH, DH, FT = 16, 64, 16
HPC = 2            # heads per core
NCORES = 8
WINDOW = 64
EPS_NORM = 1e-6
EPS_LIN = 1e-6
SUB = 128          # position sub-chunk (partition tile)
NSUB = P // SUB    # 32
SC = 256           # linear-attention scan chunk
NSC_B = T // SC    # 8 scan chunks per (b,h) sequence
QK_SCALE = 1.0 / math.sqrt(DH)
QUAD_PRE = 2.0 ** (-0.5)


def _fix_tile_drain():
    """walrus here accepts only 1 sync-wait on the Tile tail drain; spread
    the global-clock waits over sequencer nop carriers."""
    if getattr(tile.TileContext, "_drain_fix", False):
        return
    from concourse.tile import ScopedClock

    def _patched(self, tick_clock, wait_clock):
        nc = self.nc
        carriers = [nc.sync.nop(nofuse=True) for _ in range(30)]
        drain_inst = nc.sync.drain()
        wait_clock.add_sem_waits(
            drain_inst.ins, ScopedClock({None: tick_clock.global_clock})
        )
        si = drain_inst.ins.sync_info
        waits = list(si.on_wait) if si is not None else []
        if len(waits) > 1:
            keep, rest = waits[:1], waits[1:]
            assert len(rest) <= len(carriers), f"too many waits: {len(waits)}"
            for c, w in zip(carriers, rest):
                c.ins.sync_info = mybir.SyncInfo(on_wait=[w], on_update=[])
            drain_inst.ins.sync_info = mybir.SyncInfo(
                on_wait=keep, on_update=list(si.on_update)
            )
        nc.all_engine_barrier()
        assert self.sems is not None
        popped = nc._tile_sem_poison_stack.pop()
        assert popped is self._sem_poison
        nc.clear_and_free_semaphores(list(self.sems.allocated().values()))
        nc.all_engine_barrier()

    tile.TileContext._drain_and_barrier = _patched
    tile.TileContext._drain_fix = True


def _split_excess_waits(nc, limit=1):
    """walrus in this container rejects instructions with more than one
    embedded sync-wait; hoist excess waits onto preceding same-engine nops."""
    n = 0
    for f in nc.m.functions:
        for b in f.blocks:
            insts = b.instructions
            out = []
            changed = False
            for ins in insts:
                si = ins.sync_info
                waits = list(si.on_wait) if si is not None else []
                if len(waits) > limit:
                    changed = True
                    for w in waits[:-limit]:
                        n += 1
                        out.append(mybir.InstNoOp(
                            name=f"waitnop-{n}", engine=ins.engine,
                            bass_nofuse=True,
                            sync_info=mybir.SyncInfo(on_wait=[w],
                                                     on_update=[])))
                    ins.sync_info = mybir.SyncInfo(
                        on_wait=waits[-limit:], on_update=list(si.on_update))
                out.append(ins)
            if changed:
                b.instructions = out
    return n


def build_bass():
    _fix_tile_drain()
    nc = bass.Bass()
    dram = {}
    for name, shape in [
        ("xT", [D, P]), ("wq", [D, 128]), ("wk", [D, 128]), ("wv", [D, 128]),
        ("wfq", [128, 49]), ("wfk", [128, 49]),
        ("wqf1", [128, FT]), ("wkf1", [128, FT]),
        ("w1", [128, D]), ("w2", [128, D]),
        ("mtri", [128, 128]), ("mwd", [128, 128]), ("mwp", [128, 128]),
        ("onesP", [1, P]),
        ("ident", [128, 128]),
    ]:
        dram[name] = nc.dram_tensor(name, shape, BF16, kind="ExternalInput")
    dram["out"] = nc.dram_tensor("out", [P, D], BF16, kind="ExternalOutput")
    dram["scr1"] = nc.dram_tensor("scr1", [P], F32)
    dram["scr2"] = nc.dram_tensor("scr2", [P], F32)
    with tile.TileContext(nc) as tc:
        _emit(nc, tc, dram)
    _split_excess_waits(nc)
    return nc


def _emit(nc, tc, dram):
    from contextlib import ExitStack

    with ExitStack() as ctx:
        const = ctx.enter_context(tc.tile_pool(name="const", bufs=1))
        big = ctx.enter_context(tc.tile_pool(name="big", bufs=1))
        work = ctx.enter_context(tc.tile_pool(name="work", bufs=4))

        # ---- constants -----------------------------------------------
        cs = {}
        for name in ("ident", "mtri", "mwd", "mwp",
                     "wfq", "wfk", "wqf1", "wkf1", "w1", "w2"):
            d = dram[name]
            t_ = const.tile(list(d.shape), BF16, tag=name)
            nc.sync.dma_start(t_[:], d[:])
            cs[name] = t_
        for name in ("wq", "wk", "wv"):
            d = dram[name]
            t_ = const.tile([128, 8 * 128], BF16, tag=name)
            for kk in range(8):
                nc.sync.dma_start(t_[:, kk * 128:(kk + 1) * 128],
                                  d[kk * 128:(kk + 1) * 128, :])
            cs[name] = t_
        ones_col_b = const.tile([128, 1], BF16, tag="ocb")
        nc.gpsimd.memset(ones_col_b[:], 1.0)
        ones64_f = const.tile([1, 64], F32, tag="o64")
        nc.gpsimd.memset(ones64_f[:], 1.0)
        ones128_f = const.tile([1, 128], F32, tag="o128")
        nc.gpsimd.memset(ones128_f[:], 1.0)
        epsn_col = const.tile([128, 1], F32, tag="epsn")
        nc.gpsimd.memset(epsn_col[:], EPS_NORM)

        # ---- big persistent tiles ------------------------------------
        qT = big.tile([128, P], BF16, tag="qT")
        kT = big.tile([128, P], BF16, tag="kT")
        Vt = big.tile([128, NSUB * 130], BF16, tag="Vt")
        vT = big.tile([128, P], BF16, tag="vT")
        qfT = big.tile([64, P], BF16, tag="qfT")   # rows 16, 48 = ones
        kfT = big.tile([64, P], BF16, tag="kfT")
        catL = big.tile([128, P], BF16, tag="catL")
        catW = big.tile([128, P], BF16, tag="catW")
        r32 = big.tile([128, NSUB], F32, tag="r32")
        r_row = big.tile([1, P], F32, tag="rrow")
        sq_row = big.tile([1, P], F32, tag="sqrow")

        def vsl(gsub, h):
            # per sub: [v_h0 (64) | 1 | v_h1 (64) | 1]
            base = gsub * 130 + 65 * h
            return Vt[:, base:base + 65]

        def yrow(h):
            return slice(0, 64), slice(64, 65)

        with tc.tile_pool(name="xp", bufs=1) as xp:
            xt_sb = xp.tile([128, 8 * P], BF16, tag="xt")
            xv = [xt_sb[:, kk * P:(kk + 1) * P] for kk in range(8)]
            for qq in range(4):
                csl = slice(qq * (P // 4), (qq + 1) * (P // 4))
                for kk in range(8):
                    nc.sync.dma_start(xv[kk][:, csl],
                                      dram["xT"][kk * 128:(kk + 1) * 128,
                                                 csl])

            # ---- rmsnorm scale r -------------------------------------
            with tc.tile_pool(name="psq", bufs=1, space="PSUM") as psq:
                for pc in range(8):
                    sl = slice(pc * 512, (pc + 1) * 512)
                    sq_ps = psq.tile([1, 512], F32, tag="sqps")
                    for kk in range(8):
                        sqt = work.tile([128, 512], BF16, tag="sq")
                        src = xv[kk][:, sl]
                        if kk % 2 == 0:
                            nc.scalar.activation(sqt[:], src, AF.Square)
                        else:
                            nc.vector.tensor_tensor(sqt[:], src, src, ALU.mult)
                        nc.tensor.matmul(sq_ps[:], ones_col_b[:], sqt[:],
                                         start=(kk == 0), stop=(kk == 7))
                    nc.scalar.copy(sq_row[0:1, sl], sq_ps[:])
            # (1,P) -> (128,32) via DRAM bounce: r32[o, s] = row[s*128+o]
            nc.sync.dma_start(dram["scr1"][:], sq_row[:])
            nc.sync.dma_start(
                r32[:], dram["scr1"][:].rearrange("(s o) -> o s", o=128))
            nc.scalar.activation(r32[:], r32[:], AF.Sqrt,
                                 bias=epsn_col[:], scale=1.0 / D)
            nc.vector.reciprocal(r32[:], r32[:])
            nc.sync.dma_start(
                dram["scr2"][:].rearrange("(s o) -> o s", o=128), r32[:])
            nc.sync.dma_start(r_row[:], dram["scr2"][:])

            # ---- q/k projections (d-part) ----------------------------
            wq8 = [cs["wq"][:, kk * 128:(kk + 1) * 128] for kk in range(8)]
            wk8 = [cs["wk"][:, kk * 128:(kk + 1) * 128] for kk in range(8)]
            wv8 = [cs["wv"][:, kk * 128:(kk + 1) * 128] for kk in range(8)]
            with tc.tile_pool(name="ppj", bufs=3, space="PSUM") as ppj:
                for pc in range(8):
                    sl = slice(pc * 512, (pc + 1) * 512)
                    rb_ps = ppj.tile([128, 512], F32, tag="rb")
                    nc.tensor.matmul(rb_ps[:], ones128_f[:], r_row[0:1, sl],
                                     start=True, stop=True)
                    rb_sb = work.tile([128, 512], F32, tag="rbsb")
                    nc.scalar.copy(rb_sb[:], rb_ps[:])
                    for dst, w8 in ((qT, wq8), (kT, wk8), (vT, wv8)):
                        pj = ppj.tile([128, 512], F32, tag="pj")
                        for kk in range(8):
                            nc.tensor.matmul(pj[:], w8[kk], xv[kk][:, sl],
                                             start=(kk == 0), stop=(kk == 7))
                        nc.vector.tensor_tensor(dst[:, sl], pj[:], rb_sb[:],
                                                ALU.mult)

            # ---- V pos-part via DMA transpose of vT ------------------
            for s in range(NSUB):
                sl = slice(s * SUB, (s + 1) * SUB)
                for h in range(HPC):
                    nc.sync.dma_start_transpose(
                        Vt[:, s * 130 + h * 65:s * 130 + h * 65 + 64],
                        vT[h * DH:(h + 1) * DH, sl])
                    nc.gpsimd.memset(
                        Vt[:, s * 130 + h * 65 + 64:s * 130 + h * 65 + 65],
                        1.0)

        # ---- qfT / kfT (17-part per head, rows 16/33 ones) -----------
        with tc.tile_pool(name="pf", bufs=2, space="PSUM") as pf:
            for pc in range(8):
                sl = slice(pc * 512, (pc + 1) * 512)
                for dst, wf, src in ((qfT, cs["wfq"], qT), (kfT, cs["wfk"], kT)):
                    fp = pf.tile([49, 512], F32, tag="fp")
                    nc.tensor.matmul(fp[:], wf[:], src[:, sl],
                                     start=True, stop=True)
                    nc.vector.tensor_copy(dst[0:49, sl], fp[:])
        nc.sync.dma_start(qfT[16:17, :], dram["onesP"][:])
        nc.sync.dma_start(qfT[48:49, :], dram["onesP"][:])

        # ---- linear attention scan -----------------------------------
        with tc.tile_pool(name="pkv", bufs=1, space="PSUM") as pkv, \
             tc.tile_pool(name="psc", bufs=1, space="PSUM") as psc, \
             tc.tile_pool(name="psp", bufs=2, space="PSUM") as psp2, \
             tc.tile_pool(name="pyt", bufs=2, space="PSUM") as pyt, \
             tc.tile_pool(name="phi", bufs=17) as phip:
            for b in range(B):
                for h in range(HPC):
                    hd = slice(h * DH, (h + 1) * DH)
                    h17 = slice(h * 32, h * 32 + 17)
                    h16 = slice(h * 32, h * 32 + 16)
                    kvq = pkv.tile([128, 130], F32, tag="kvq")
                    kvlo = pkv.tile([17, 65], F32, tag="kvlo")
                    kvq_sb = work.tile([128, 130], BF16, tag="kvqs")
                    kvlo_sb = work.tile([49, 65], BF16, tag="kvlos")
                    all_q, all_k = [], []
                    for sc in range(NSC_B):
                        p0 = b * T + sc * SC
                        quads_q, quads_k = [], []
                        all_q.append(quads_q)
                        all_k.append(quads_k)
                        for cb in range(2):
                            sl = slice(p0 + cb * 128, p0 + (cb + 1) * 128)
                            qk_ps = psc.tile([128, 32], F32, tag="qkps")
                            nc.tensor.matmul(qk_ps[:, 0:16], qT[hd, sl],
                                             cs["wqf1"][hd, :], start=True,
                                             stop=True)
                            nc.tensor.matmul(qk_ps[:, 16:32], kT[hd, sl],
                                             cs["wkf1"][hd, :], start=True,
                                             stop=True)
                            qfp = phip.tile([128, FT], BF16, tag="qfp")
                            klin = phip.tile([128, 17], BF16, tag="klin")
                            # host folds 2^+0.5 into wfq, 2^-0.5 into wfk:
                            # klin doubles as the quad-scaled kf.
                            nc.scalar.activation(qfp[:], qk_ps[:, 0:16],
                                                 AF.Copy, bias=0.0,
                                                 scale=0.5)
                            nc.scalar.copy(klin[:, 0:16], qk_ps[:, 16:32])
                            nc.gpsimd.memset(klin[:, 16:17], 1.0)
                            quad_q = phip.tile([128, 256], BF16, tag="qq")
                            quad_k = phip.tile([128, 256], BF16, tag="qk")
                            for qd, fsrc in ((quad_q, qfp[:]),
                                             (quad_k, klin[:, 0:16])):
                                g1 = fsrc.unsqueeze(2).broadcast_to(
                                    (128, FT, FT))
                                g2 = fsrc.unsqueeze(1).broadcast_to(
                                    (128, FT, FT))
                                nc.gpsimd.tensor_tensor(
                                    qd[:].rearrange("p (i j) -> p i j", i=FT),
                                    g1, g2, ALU.mult)
                            q1sb = phip.tile([128, 128], BF16, tag="q1sb")
                            q2sb = phip.tile([128, 128], BF16, tag="q2sb")
                            for half, qsb in ((0, q1sb), (1, q2sb)):
                                nc.sync.dma_start_transpose(
                                    qsb[:],
                                    quad_q[:, half * 128:(half + 1) * 128])
                            quads_q.append((q1sb, q2sb))
                            quads_k.append((quad_k, klin))

                    for sc in range(NSC_B):
                        p0 = b * T + sc * SC
                        gs0 = p0 // SUB
                        quads_q = all_q[sc]
                        quads_k = all_k[sc]
                        yts = []
                        for cb in range(2):
                            sl = slice(p0 + cb * 128, p0 + (cb + 1) * 128)
                            yt = pyt.tile([65, 128], F32, tag="yt")
                            ops = []
                            for sb in range(cb + 1):
                                ssl = slice(p0 + sb * 128,
                                            p0 + (sb + 1) * 128)
                                s_ps = psp2.tile([128, 128], F32, tag="sps")
                                nc.tensor.matmul(s_ps[:], kfT[h16, ssl],
                                                 qfT[h16, sl],
                                                 start=True, stop=True)
                                a_sb = work.tile([128, 128], BF16, tag="asb")
                                nc.scalar.activation(a_sb[:], s_ps[:],
                                                     AF.Square,
                                                     bias=1.0, scale=0.5)
                                if sb == cb:
                                    nc.vector.tensor_tensor(
                                        a_sb[:], a_sb[:], cs["mtri"][:],
                                        ALU.mult)
                                ops.append((vsl(gs0 + sb, h), a_sb[:]))
                            if sc > 0:
                                q1sb, q2sb = quads_q[cb]
                                ops.append((kvq_sb[:, 0:65], q1sb[:]))
                                ops.append((kvq_sb[:, 65:130], q2sb[:]))
                                ops.append((kvlo_sb[h17, :], qfT[h17, sl]))
                            for i, (lt, rt) in enumerate(ops):
                                nc.tensor.matmul(yt[:], lt, rt,
                                                 start=(i == 0),
                                                 stop=(i == len(ops) - 1))
                            yts.append(yt)

                        for cb in range(2):
                            va = vsl(gs0 + cb, h)
                            quad_k, klin = quads_k[cb]
                            st = (sc == 0 and cb == 0)
                            sp = (sc == NSC_B - 1 and cb == 1)
                            nc.tensor.matmul(kvq[:, 0:65], quad_k[:, 0:128],
                                             va, start=st, stop=sp)
                            nc.tensor.matmul(kvq[:, 65:130],
                                             quad_k[:, 128:256], va,
                                             start=st, stop=sp)
                            nc.tensor.matmul(kvlo[:], klin[:], va,
                                             start=st, stop=sp)
                        if sc < NSC_B - 1:
                            nc.vector.tensor_copy(kvq_sb[:], kvq[:])
                            nc.vector.tensor_copy(kvlo_sb[h17, :], kvlo[:])

                        sl2 = slice(p0, p0 + SC)
                        ysl, zsl = yrow(h)
                        zi = work.tile([1, 256], F32, tag="zi")
                        nc.vector.reciprocal(zi[0:1, 0:128],
                                             yts[0][zsl, :])
                        nc.vector.reciprocal(zi[0:1, 128:256],
                                             yts[1][zsl, :])
                        zb = psc.tile([64, 256], F32, tag="zb")
                        nc.tensor.matmul(zb[:], ones64_f[:], zi[:],
                                         start=True, stop=True)
                        ysb = work.tile([64, 256], BF16, tag="ysb")
                        nc.scalar.copy(ysb[:, 0:128], yts[0][ysl, :])
                        nc.scalar.copy(ysb[:, 128:256], yts[1][ysl, :])
                        nc.vector.tensor_tensor(catL[hd, sl2], ysb[:], zb[:],
                                                ALU.mult)

        # ---- sliding window attention --------------------------------
        with tc.tile_pool(name="pst", bufs=3, space="PSUM") as pst, \
             tc.tile_pool(name="pyw", bufs=3, space="PSUM") as pyw, \
             tc.tile_pool(name="pzw", bufs=2, space="PSUM") as pzw:
            for b in range(B):
                for c in range(T // SUB):
                    p0 = b * T + c * SUB
                    sl = slice(p0, p0 + SUB)
                    for h in range(HPC):
                        hd = slice(h * DH, (h + 1) * DH)
                        ytw = pyw.tile([65, 128], F32, tag="ytw")
                        sblocks = [c] if c == 0 else [c - 1, c]
                        for i, sb in enumerate(sblocks):
                            ssl = slice(b * T + sb * SUB,
                                        b * T + (sb + 1) * SUB)
                            st_ps = pst.tile([128, 128], F32, tag="stps")
                            nc.tensor.matmul(st_ps[:], kT[hd, ssl],
                                             qT[hd, sl], start=True,
                                             stop=True)
                            pexp = work.tile([128, 128], BF16, tag="pexp")
                            nc.scalar.activation(pexp[:], st_ps[:], AF.Exp,
                                                 bias=0.0, scale=QK_SCALE)
                            msk = cs["mwd"] if sb == c else cs["mwp"]
                            nc.vector.tensor_tensor(pexp[:], pexp[:], msk[:],
                                                    ALU.mult)
                            nc.tensor.matmul(
                                ytw[:], vsl(b * (T // SUB) + sb, h), pexp[:],
                                start=(i == 0),
                                stop=(i == len(sblocks) - 1))
                        ziw = work.tile([1, 128], F32, tag="ziw")
                        nc.vector.reciprocal(ziw[:], ytw[64:65, :])
                        zbw = pzw.tile([64, 128], F32, tag="zbw")
                        nc.tensor.matmul(zbw[:], ones64_f[:], ziw[:],
                                         start=True, stop=True)
                        ywsb = work.tile([64, 128], BF16, tag="ywsb")
                        nc.scalar.copy(ywsb[:], ytw[0:64, :])
                        nc.vector.tensor_tensor(catW[hd, sl], ywsb[:],
                                                zbw[:], ALU.mult)

        # ---- out-projection ------------------------------------------
        with tc.tile_pool(name="pop", bufs=3, space="PSUM") as pop, \
             tc.tile_pool(name="outp", bufs=3) as outp:
            for s in range(NSUB):
                sl = slice(s * SUB, (s + 1) * SUB)
                op = pop.tile([128, D], F32, tag="op")
                for hf in range(2):
                    c512 = slice(hf * 512, (hf + 1) * 512)
                    nc.tensor.matmul(op[:, c512], catL[:, sl],
                                     cs["w1"][:, c512], start=True, stop=False)
                    nc.tensor.matmul(op[:, c512], catW[:, sl],
                                     cs["w2"][:, c512], start=False, stop=True)
                ob = outp.tile([128, D], BF16, tag="ob")
                nc.scalar.copy(ob[:], op[:])
                nc.gpsimd.dma_start(dram["out"][sl, :], ob[:])


_NC_CACHE = None


def _get_nc():
    global _NC_CACHE
    if _NC_CACHE is None:
        _NC_CACHE = build_bass()
    return _NC_CACHE


def _host_prep(x, norm_w, Wq, Wk, Wv, Wqf, Wkf, Wout):
    xp = np.ascontiguousarray(x.reshape(P, D).T).astype(BF)
    nw = norm_w.astype(np.float64)
    wq_f = nw[:, None] * Wq.astype(np.float64)
    wk_f = nw[:, None] * Wk.astype(np.float64)
    wv_f = nw[:, None] * Wv.astype(np.float64)

    si = np.arange(128)[:, None]
    ci = np.arange(128)[None, :]
    mtri = (si <= ci).astype(np.float32)
    mwd = ((si <= ci) & (si >= ci - WINDOW)).astype(np.float32)
    mwp = (si >= ci + WINDOW).astype(np.float32)

    sq2 = math.sqrt(2.0)
    wfq = np.zeros((128, 49), np.float32)
    wfq[0:64, 0:16] = Wqf * sq2
    wfq[64:128, 32:48] = Wqf * sq2
    wfk = np.zeros((128, 49), np.float32)
    wfk[0:64, 0:16] = Wkf / sq2
    wfk[64:128, 32:48] = Wkf / sq2

    in_maps = []
    for c in range(NCORES):
        csl = slice(c * 128, (c + 1) * 128)
        in_maps.append({
            "xT": xp,
            "wq": wq_f[:, csl].astype(BF),
            "wk": wk_f[:, csl].astype(BF),
            "wv": wv_f[:, csl].astype(BF),
            "wfq": wfq.astype(BF),
            "wfk": wfk.astype(BF),
            "wqf1": (np.vstack([Wqf, Wqf]) * sq2).astype(BF),
            "wkf1": (np.vstack([Wkf, Wkf]) / sq2).astype(BF),
            "w1": Wout[csl, :].astype(BF),
            "w2": Wout[1024 + c * 128:1024 + (c + 1) * 128, :].astype(BF),
            "mtri": mtri.astype(BF),
            "mwd": mwd.astype(BF),
            "mwp": mwp.astype(BF),
            "ident": np.eye(128, dtype=np.float32).astype(BF),
            "onesP": np.ones((1, P), np.float32).astype(BF),
        })
    return in_maps


def kernel(x, norm_w, Wq, Wk, Wv, Wqf, Wkf, Wout) -> np.ndarray:
    x = np.asarray(x, np.float32)
    in_maps = _host_prep(
        x, np.asarray(norm_w, np.float32), np.asarray(Wq, np.float32),
        np.asarray(Wk, np.float32), np.asarray(Wv, np.float32),
        np.asarray(Wqf, np.float32), np.asarray(Wkf, np.float32),
        np.asarray(Wout, np.float32))
    nc = _get_nc()
    res = run_bass_kernel_spmd(nc, in_maps, list(range(NCORES)))
    acc = np.zeros((P, D), np.float32)
    for c in range(NCORES):
        acc += res.results[c]["out"].astype(np.float32)
    return (x.reshape(P, D) + acc).reshape(B, T, D).astype(np.float32)


# revision 41
# speedup vs baseline: 1.2205x; 1.0568x over previous
"""BasedAttention Trainium2 kernel — nn_BasedAttention_82214263980185.

Head-sharded across 8 NeuronCores (2 heads/core): column-parallel QKV,
per-head taylor linear attention (factorized phi) + banded sliding-window
attention, row-parallel out-proj with host-side partial reduction.

Math notes:
  - reference phi(x) = [1, x, tri-scaled quad] gives
    phi(q).phi(k) = 1 + s + 0.25 s^2  (s = qf.kf).  We use the equivalent
    full-outer 256-feature quad block scaled 2^-0.25 per side plus
    [x, ones]: identical inner products, rectangular construction.
  - Intra-chunk scores: A = (1 + 0.5 s)^2 = 1 + s + 0.25 s^2 directly.
  - rmsnorm: norm_w folds into QKV weights on host; the per-row 1/rms
    factor r applies to q, k, v after projection (all linear in r).
"""

import math
import sys

for _p in ("/opt/trn_rl_repo",):
    if _p not in sys.path:
        sys.path.insert(0, _p)

import numpy as np
import ml_dtypes

import concourse.bass as bass
import concourse.mybir as mybir
import concourse.tile as tile
from concourse.bass_utils import run_bass_kernel_spmd

F32 = mybir.dt.float32
BF16 = mybir.dt.bfloat16
AF = mybir.ActivationFunctionType
ALU = mybir.AluOpType
BF = ml_dtypes.bfloat16

B, T, D = 2, 2048, 1024
P = B * T          # 4096 positions
NH, DH, FT = 16, 64, 16
HPC = 2            # heads per core
NCORES = 8
WINDOW = 64
EPS_NORM = 1e-6
EPS_LIN = 1e-6
SUB = 128          # position sub-chunk (partition tile)
NSUB = P // SUB    # 32
SC = 256           # linear-attention scan chunk
NSC_B = T // SC    # 8 scan chunks per (b,h) sequence
QK_SCALE = 1.0 / math.sqrt(DH)
QUAD_PRE = 2.0 ** (-0.5)


def _fix_tile_drain():
    """walrus here accepts only 1 sync-wait on the Tile tail drain; spread
    the global-clock waits over sequencer nop carriers."""
    if getattr(tile.TileContext, "_drain_fix", False):
        return
    from concourse.tile import ScopedClock

    def _patched(self, tick_clock, wait_clock):
        nc = self.nc
        carriers = [nc.sync.nop(nofuse=True) for _ in range(30)]
        drain_inst = nc.sync.drain()
        wait_clock.add_sem_waits(
            drain_inst.ins, ScopedClock({None: tick_clock.global_clock})
        )
        si = drain_inst.ins.sync_info
        waits = list(si.on_wait) if si is not None else []
        if len(waits) > 1:
            keep, rest = waits[:1], waits[1:]
            assert len(rest) <= len(carriers), f"too many waits: {len(waits)}"
            for c, w in zip(carriers, rest):
                c.ins.sync_info = mybir.SyncInfo(on_wait=[w], on_update=[])
            drain_inst.ins.sync_info = mybir.SyncInfo(
                on_wait=keep, on_update=list(si.on_update)
            )
        nc.all_engine_barrier()
        assert self.sems is not None
        popped = nc._tile_sem_poison_stack.pop()
        assert popped is self._sem_poison
        nc.clear_and_free_semaphores(list(self.sems.allocated().values()))
        nc.all_engine_barrier()

    tile.TileContext._drain_and_barrier = _patched
    tile.TileContext._drain_fix = True


def _split_excess_waits(nc, limit=1):
    """walrus in this container rejects instructions with more than one
    embedded sync-wait; hoist excess waits onto preceding same-engine nops."""
    n = 0
    for f in nc.m.functions:
        for b in f.blocks:
            insts = b.instructions
            out = []
            changed = False
            for ins in insts:
                si = ins.sync_info
                waits = list(si.on_wait) if si is not None else []
                if len(waits) > limit:
                    changed = True
                    for w in waits[:-limit]:
                        n += 1
                        out.append(mybir.InstNoOp(
                            name=f"waitnop-{n}", engine=ins.engine,
                            bass_nofuse=True,
                            sync_info=mybir.SyncInfo(on_wait=[w],
                                                     on_update=[])))
                    ins.sync_info = mybir.SyncInfo(
                        on_wait=waits[-limit:], on_update=list(si.on_update))
                out.append(ins)
            if changed:
                b.instructions = out
    return n


def build_bass():
    _fix_tile_drain()
    nc = bass.Bass()
    dram = {}
    for name, shape in [
        ("xT", [D, P]), ("wq", [D, 128]), ("wk", [D, 128]), ("wv", [D, 128]),
        ("wfq", [128, 49]), ("wfk", [128, 49]),
        ("wqf1", [128, FT]), ("wkf1", [128, FT]),
        ("w1", [128, D]), ("w2", [128, D]),
        ("mtri", [128, 128]), ("mwd", [128, 128]), ("mwp", [128, 128]),
        ("onesP", [1, P]),
        ("ident", [128, 128]),
    ]:
        dram[name] = nc.dram_tensor(name, shape, BF16, kind="ExternalInput")
    dram["out"] = nc.dram_tensor("out", [P, D], BF16, kind="ExternalOutput")
    dram["scr1"] = nc.dram_tensor("scr1", [P], F32)
    dram["scr2"] = nc.dram_tensor("scr2", [P], F32)
    with tile.TileContext(nc) as tc:
        _emit(nc, tc, dram)
    _split_excess_waits(nc)
    return nc


def _emit(nc, tc, dram):
    from contextlib import ExitStack

    with ExitStack() as ctx:
        const = ctx.enter_context(tc.tile_pool(name="const", bufs=1))
        big = ctx.enter_context(tc.tile_pool(name="big", bufs=1))
        work = ctx.enter_context(tc.tile_pool(name="work", bufs=4))

        # ---- constants -----------------------------------------------
        cs = {}
        for name in ("ident", "mtri", "mwd", "mwp",
                     "wfq", "wfk", "wqf1", "wkf1", "w1", "w2"):
            d = dram[name]
            t_ = const.tile(list(d.shape), BF16, tag=name)
            nc.sync.dma_start(t_[:], d[:])
            cs[name] = t_
        for name in ("wq", "wk", "wv"):
            d = dram[name]
            t_ = const.tile([128, 8 * 128], BF16, tag=name)
            for kk in range(8):
                nc.sync.dma_start(t_[:, kk * 128:(kk + 1) * 128],
                                  d[kk * 128:(kk + 1) * 128, :])
            cs[name] = t_
        ones_col_b = const.tile([128, 1], BF16, tag="ocb")
        nc.gpsimd.memset(ones_col_b[:], 1.0)
        ones64_f = const.tile([1, 64], F32, tag="o64")
        nc.gpsimd.memset(ones64_f[:], 1.0)
        ones128_f = const.tile([1, 128], F32, tag="o128")
        nc.gpsimd.memset(ones128_f[:], 1.0)
        epsn_col = const.tile([128, 1], F32, tag="epsn")
        nc.gpsimd.memset(epsn_col[:], EPS_NORM)

        # ---- big persistent tiles ------------------------------------
        qT = big.tile([128, P], BF16, tag="qT")
        kT = big.tile([128, P], BF16, tag="kT")
        Vt = big.tile([128, NSUB * 130], BF16, tag="Vt")
        vT = big.tile([128, P], BF16, tag="vT")
        qfT = big.tile([64, P], BF16, tag="qfT")   # rows 16, 48 = ones
        kfT = big.tile([64, P], BF16, tag="kfT")
        catL = big.tile([128, P], BF16, tag="catL")
        catW = big.tile([128, P], BF16, tag="catW")
        r32 = big.tile([128, NSUB], F32, tag="r32")
        r_row = big.tile([1, P], F32, tag="rrow")
        sq_row = big.tile([1, P], F32, tag="sqrow")

        def vsl(gsub, h):
            # per sub: [v_h0 (64) | 1 | v_h1 (64) | 1]
            base = gsub * 130 + 65 * h
            return Vt[:, base:base + 65]

        def yrow(h):
            return slice(0, 64), slice(64, 65)

        with tc.tile_pool(name="xp", bufs=1) as xp:
            xt_sb = xp.tile([128, 8 * P], BF16, tag="xt")
            xv = [xt_sb[:, kk * P:(kk + 1) * P] for kk in range(8)]
            for qq in range(4):
                csl = slice(qq * (P // 4), (qq + 1) * (P // 4))
                for kk in range(8):
                    nc.sync.dma_start(xv[kk][:, csl],
                                      dram["xT"][kk * 128:(kk + 1) * 128,
                                                 csl])

            # ---- rmsnorm scale r -------------------------------------
            with tc.tile_pool(name="psq", bufs=1, space="PSUM") as psq:
                for pc in range(8):
                    sl = slice(pc * 512, (pc + 1) * 512)
                    sq_ps = psq.tile([1, 512], F32, tag="sqps")
                    for kk in range(8):
                        sqt = work.tile([128, 512], BF16, tag="sq")
                        src = xv[kk][:, sl]
                        if kk % 2 == 0:
                            nc.scalar.activation(sqt[:], src, AF.Square)
                        else:
                            nc.vector.tensor_tensor(sqt[:], src, src, ALU.mult)
                        nc.tensor.matmul(sq_ps[:], ones_col_b[:], sqt[:],
                                         start=(kk == 0), stop=(kk == 7))
                    nc.scalar.copy(sq_row[0:1, sl], sq_ps[:])
            # (1,P) -> (128,32) via DRAM bounce: r32[o, s] = row[s*128+o]
            nc.sync.dma_start(dram["scr1"][:], sq_row[:])
            nc.sync.dma_start(
                r32[:], dram["scr1"][:].rearrange("(s o) -> o s", o=128))
            nc.scalar.activation(r32[:], r32[:], AF.Sqrt,
                                 bias=epsn_col[:], scale=1.0 / D)
            nc.vector.reciprocal(r32[:], r32[:])
            nc.sync.dma_start(
                dram["scr2"][:].rearrange("(s o) -> o s", o=128), r32[:])
            nc.sync.dma_start(r_row[:], dram["scr2"][:])

            # ---- q/k projections (d-part) ----------------------------
            wq8 = [cs["wq"][:, kk * 128:(kk + 1) * 128] for kk in range(8)]
            wk8 = [cs["wk"][:, kk * 128:(kk + 1) * 128] for kk in range(8)]
            wv8 = [cs["wv"][:, kk * 128:(kk + 1) * 128] for kk in range(8)]
            with tc.tile_pool(name="ppj", bufs=3, space="PSUM") as ppj:
                for pc in range(8):
                    sl = slice(pc * 512, (pc + 1) * 512)
                    rb_ps = ppj.tile([128, 512], F32, tag="rb")
                    nc.tensor.matmul(rb_ps[:], ones128_f[:], r_row[0:1, sl],
                                     start=True, stop=True)
                    rb_sb = work.tile([128, 512], F32, tag="rbsb")
                    nc.scalar.copy(rb_sb[:], rb_ps[:])
                    for dst, w8 in ((qT, wq8), (kT, wk8), (vT, wv8)):
                        pj = ppj.tile([128, 512], F32, tag="pj")
                        for kk in range(8):
                            nc.tensor.matmul(pj[:], w8[kk], xv[kk][:, sl],
                                             start=(kk == 0), stop=(kk == 7))
                        nc.vector.tensor_tensor(dst[:, sl], pj[:], rb_sb[:],
                                                ALU.mult)

            # ---- V pos-part via DMA transpose of vT ------------------
            for s in range(NSUB):
                sl = slice(s * SUB, (s + 1) * SUB)
                for h in range(HPC):
                    nc.sync.dma_start_transpose(
                        Vt[:, s * 130 + h * 65:s * 130 + h * 65 + 64],
                        vT[h * DH:(h + 1) * DH, sl])
                    nc.gpsimd.memset(
                        Vt[:, s * 130 + h * 65 + 64:s * 130 + h * 65 + 65],
                        1.0)

        # ---- qfT / kfT (17-part per head, rows 16/33 ones) -----------
        with tc.tile_pool(name="pf", bufs=3, space="PSUM") as pf:
            for pc in range(8):
                sl = slice(pc * 512, (pc + 1) * 512)
                for dst, wf, src in ((qfT, cs["wfq"], qT), (kfT, cs["wfk"], kT)):
                    fp = pf.tile([49, 512], F32, tag="fp")
                    nc.tensor.matmul(fp[:], wf[:], src[:, sl],
                                     start=True, stop=True)
                    nc.vector.tensor_copy(dst[0:49, sl], fp[:])
        nc.sync.dma_start(qfT[16:17, :], dram["onesP"][:])
        nc.sync.dma_start(qfT[48:49, :], dram["onesP"][:])

        # ---- linear attention scan -----------------------------------
        with tc.tile_pool(name="pkv", bufs=1, space="PSUM") as pkv, \
             tc.tile_pool(name="psc", bufs=1, space="PSUM") as psc, \
             tc.tile_pool(name="psp", bufs=2, space="PSUM") as psp2, \
             tc.tile_pool(name="pyt", bufs=2, space="PSUM") as pyt, \
             tc.tile_pool(name="phi", bufs=17) as phip:
            for b in range(B):
                for h in range(HPC):
                    hd = slice(h * DH, (h + 1) * DH)
                    h17 = slice(h * 32, h * 32 + 17)
                    h16 = slice(h * 32, h * 32 + 16)
                    kvq = pkv.tile([128, 130], F32, tag="kvq")
                    kvlo = pkv.tile([17, 65], F32, tag="kvlo")
                    kvq_sb = work.tile([128, 130], BF16, tag="kvqs")
                    kvlo_sb = work.tile([49, 65], BF16, tag="kvlos")
                    all_q, all_k = [], []
                    for sc in range(NSC_B):
                        p0 = b * T + sc * SC
                        quads_q, quads_k = [], []
                        all_q.append(quads_q)
                        all_k.append(quads_k)
                        for cb in range(2):
                            sl = slice(p0 + cb * 128, p0 + (cb + 1) * 128)
                            qk_ps = psc.tile([128, 32], F32, tag="qkps")
                            nc.tensor.matmul(qk_ps[:, 0:16], qT[hd, sl],
                                             cs["wqf1"][hd, :], start=True,
                                             stop=True)
                            nc.tensor.matmul(qk_ps[:, 16:32], kT[hd, sl],
                                             cs["wkf1"][hd, :], start=True,
                                             stop=True)
                            qfp = phip.tile([128, FT], BF16, tag="qfp")
                            klin = phip.tile([128, 17], BF16, tag="klin")
                            # host folds 2^+0.5 into wfq, 2^-0.5 into wfk:
                            # klin doubles as the quad-scaled kf.
                            nc.scalar.activation(qfp[:], qk_ps[:, 0:16],
                                                 AF.Copy, bias=0.0,
                                                 scale=0.5)
                            nc.scalar.copy(klin[:, 0:16], qk_ps[:, 16:32])
                            nc.gpsimd.memset(klin[:, 16:17], 1.0)
                            quad_q = phip.tile([128, 256], BF16, tag="qq")
                            quad_k = phip.tile([128, 256], BF16, tag="qk")
                            for qd, fsrc in ((quad_q, qfp[:]),
                                             (quad_k, klin[:, 0:16])):
                                g1 = fsrc.unsqueeze(2).broadcast_to(
                                    (128, FT, FT))
                                g2 = fsrc.unsqueeze(1).broadcast_to(
                                    (128, FT, FT))
                                nc.gpsimd.tensor_tensor(
                                    qd[:].rearrange("p (i j) -> p i j", i=FT),
                                    g1, g2, ALU.mult)
                            q1sb = phip.tile([128, 128], BF16, tag="q1sb")
                            q2sb = phip.tile([128, 128], BF16, tag="q2sb")
                            for half, qsb in ((0, q1sb), (1, q2sb)):
                                nc.sync.dma_start_transpose(
                                    qsb[:],
                                    quad_q[:, half * 128:(half + 1) * 128])
                            quads_q.append((q1sb, q2sb))
                            quads_k.append((quad_k, klin))

                    for sc in range(NSC_B):
                        p0 = b * T + sc * SC
                        gs0 = p0 // SUB
                        quads_q = all_q[sc]
                        quads_k = all_k[sc]
                        yts = []
                        for cb in range(2):
                            sl = slice(p0 + cb * 128, p0 + (cb + 1) * 128)
                            yt = pyt.tile([65, 128], F32, tag="yt")
                            ops = []
                            for sb in range(cb + 1):
                                ssl = slice(p0 + sb * 128,
                                            p0 + (sb + 1) * 128)
                                s_ps = psp2.tile([128, 128], F32, tag="sps")
                                nc.tensor.matmul(s_ps[:], kfT[h16, ssl],
                                                 qfT[h16, sl],
                                                 start=True, stop=True)
                                a_sb = work.tile([128, 128], BF16, tag="asb")
                                nc.scalar.activation(a_sb[:], s_ps[:],
                                                     AF.Square,
                                                     bias=1.0, scale=0.5)
                                if sb == cb:
                                    nc.vector.tensor_tensor(
                                        a_sb[:], a_sb[:], cs["mtri"][:],
                                        ALU.mult)
                                ops.append((vsl(gs0 + sb, h), a_sb[:]))
                            if sc > 0:
                                q1sb, q2sb = quads_q[cb]
                                ops.append((kvq_sb[:, 0:65], q1sb[:]))
                                ops.append((kvq_sb[:, 65:130], q2sb[:]))
                                ops.append((kvlo_sb[h17, :], qfT[h17, sl]))
                            for i, (lt, rt) in enumerate(ops):
                                nc.tensor.matmul(yt[:], lt, rt,
                                                 start=(i == 0),
                                                 stop=(i == len(ops) - 1))
                            yts.append(yt)

                        for cb in range(2):
                            va = vsl(gs0 + cb, h)
                            quad_k, klin = quads_k[cb]
                            st = (sc == 0 and cb == 0)
                            sp = (sc == NSC_B - 1 and cb == 1)
                            nc.tensor.matmul(kvq[:, 0:65], quad_k[:, 0:128],
                                             va, start=st, stop=sp)
                            nc.tensor.matmul(kvq[:, 65:130],
                                             quad_k[:, 128:256], va,
                                             start=st, stop=sp)
                            nc.tensor.matmul(kvlo[:], klin[:], va,
                                             start=st, stop=sp)
                        if sc < NSC_B - 1:
                            nc.vector.tensor_copy(kvq_sb[:], kvq[:])
                            nc.vector.tensor_copy(kvlo_sb[h17, :], kvlo[:])

                        sl2 = slice(p0, p0 + SC)
                        ysl, zsl = yrow(h)
                        zi = work.tile([1, 256], F32, tag="zi")
                        nc.vector.reciprocal(zi[0:1, 0:128],
                                             yts[0][zsl, :])
                        nc.vector.reciprocal(zi[0:1, 128:256],
                                             yts[1][zsl, :])
                        zb = psc.tile([64, 256], F32, tag="zb")
                        nc.tensor.matmul(zb[:], ones64_f[:], zi[:],
                                         start=True, stop=True)
                        ysb = work.tile([64, 256], BF16, tag="ysb")
                        nc.scalar.copy(ysb[:, 0:128], yts[0][ysl, :])
                        nc.scalar.copy(ysb[:, 128:256], yts[1][ysl, :])
                        nc.vector.tensor_tensor(catL[hd, sl2], ysb[:], zb[:],
                                                ALU.mult)

        # ---- sliding window attention --------------------------------
        with tc.tile_pool(name="pst", bufs=3, space="PSUM") as pst, \
             tc.tile_pool(name="pyw", bufs=3, space="PSUM") as pyw, \
             tc.tile_pool(name="pzw", bufs=2, space="PSUM") as pzw:
            for b in range(B):
                for c in range(T // SUB):
                    p0 = b * T + c * SUB
                    sl = slice(p0, p0 + SUB)
                    for h in range(HPC):
                        hd = slice(h * DH, (h + 1) * DH)
                        ytw = pyw.tile([65, 128], F32, tag="ytw")
                        sblocks = [c] if c == 0 else [c - 1, c]
                        for i, sb in enumerate(sblocks):
                            ssl = slice(b * T + sb * SUB,
                                        b * T + (sb + 1) * SUB)
                            st_ps = pst.tile([128, 128], F32, tag="stps")
                            nc.tensor.matmul(st_ps[:], kT[hd, ssl],
                                             qT[hd, sl], start=True,
                                             stop=True)
                            pexp = work.tile([128, 128], BF16, tag="pexp")
                            nc.scalar.activation(pexp[:], st_ps[:], AF.Exp,
                                                 bias=0.0, scale=QK_SCALE)
                            msk = cs["mwd"] if sb == c else cs["mwp"]
                            nc.vector.tensor_tensor(pexp[:], pexp[:], msk[:],
                                                    ALU.mult)
                            nc.tensor.matmul(
                                ytw[:], vsl(b * (T // SUB) + sb, h), pexp[:],
                                start=(i == 0),
                                stop=(i == len(sblocks) - 1))
                        ziw = work.tile([1, 128], F32, tag="ziw")
                        nc.vector.reciprocal(ziw[:], ytw[64:65, :])
                        zbw = pzw.tile([64, 128], F32, tag="zbw")
                        nc.tensor.matmul(zbw[:], ones64_f[:], ziw[:],
                                         start=True, stop=True)
                        ywsb = work.tile([64, 128], BF16, tag="ywsb")
                        nc.scalar.copy(ywsb[:], ytw[0:64, :])
                        nc.vector.tensor_tensor(catW[hd, sl], ywsb[:],
                                                zbw[:], ALU.mult)

        # ---- out-projection ------------------------------------------
        with tc.tile_pool(name="pop", bufs=3, space="PSUM") as pop, \
             tc.tile_pool(name="outp", bufs=5) as outp:
            for s in range(NSUB):
                sl = slice(s * SUB, (s + 1) * SUB)
                op = pop.tile([128, D], F32, tag="op")
                for hf in range(2):
                    c512 = slice(hf * 512, (hf + 1) * 512)
                    nc.tensor.matmul(op[:, c512], catL[:, sl],
                                     cs["w1"][:, c512], start=True, stop=False)
                    nc.tensor.matmul(op[:, c512], catW[:, sl],
                                     cs["w2"][:, c512], start=False, stop=True)
                ob = outp.tile([128, D], BF16, tag="ob")
                nc.scalar.copy(ob[:], op[:])
                nc.gpsimd.dma_start(dram["out"][sl, :], ob[:])


_NC_CACHE = None


def _get_nc():
    global _NC_CACHE
    if _NC_CACHE is None:
        _NC_CACHE = build_bass()
    return _NC_CACHE


def _host_prep(x, norm_w, Wq, Wk, Wv, Wqf, Wkf, Wout):
    xp = np.ascontiguousarray(x.reshape(P, D).T).astype(BF)
    nw = norm_w.astype(np.float64)
    wq_f = nw[:, None] * Wq.astype(np.float64)
    wk_f = nw[:, None] * Wk.astype(np.float64)
    wv_f = nw[:, None] * Wv.astype(np.float64)

    si = np.arange(128)[:, None]
    ci = np.arange(128)[None, :]
    mtri = (si <= ci).astype(np.float32)
    mwd = ((si <= ci) & (si >= ci - WINDOW)).astype(np.float32)
    mwp = (si >= ci + WINDOW).astype(np.float32)

    sq2 = math.sqrt(2.0)
    wfq = np.zeros((128, 49), np.float32)
    wfq[0:64, 0:16] = Wqf * sq2
    wfq[64:128, 32:48] = Wqf * sq2
    wfk = np.zeros((128, 49), np.float32)
    wfk[0:64, 0:16] = Wkf / sq2
    wfk[64:128, 32:48] = Wkf / sq2

    in_maps = []
    for c in range(NCORES):
        csl = slice(c * 128, (c + 1) * 128)
        in_maps.append({
            "xT": xp,
            "wq": wq_f[:, csl].astype(BF),
            "wk": wk_f[:, csl].astype(BF),
            "wv": wv_f[:, csl].astype(BF),
            "wfq": wfq.astype(BF),
            "wfk": wfk.astype(BF),
            "wqf1": (np.vstack([Wqf, Wqf]) * sq2).astype(BF),
            "wkf1": (np.vstack([Wkf, Wkf]) / sq2).astype(BF),
            "w1": Wout[csl, :].astype(BF),
            "w2": Wout[1024 + c * 128:1024 + (c + 1) * 128, :].astype(BF),
            "mtri": mtri.astype(BF),
            "mwd": mwd.astype(BF),
            "mwp": mwp.astype(BF),
            "ident": np.eye(128, dtype=np.float32).astype(BF),
            "onesP": np.ones((1, P), np.float32).astype(BF),
        })
    return in_maps


def kernel(x, norm_w, Wq, Wk, Wv, Wqf, Wkf, Wout) -> np.ndarray:
    x = np.asarray(x, np.float32)
    in_maps = _host_prep(
        x, np.asarray(norm_w, np.float32), np.asarray(Wq, np.float32),
        np.asarray(Wk, np.float32), np.asarray(Wv, np.float32),
        np.asarray(Wqf, np.float32), np.asarray(Wkf, np.float32),
        np.asarray(Wout, np.float32))
    nc = _get_nc()
    res = run_bass_kernel_spmd(nc, in_maps, list(range(NCORES)))
    acc = np.zeros((P, D), np.float32)
    for c in range(NCORES):
        acc += res.results[c]["out"].astype(np.float32)
    return (x.reshape(P, D) + acc).reshape(B, T, D).astype(np.float32)
